# revision 2
# baseline (speedup 1.0000x reference)
"""Trainium2 Bass kernel for StyleGAN2-style fused upsample(x2)+conv3x3+FIR.

Reference computation (per image):
    y1 = conv_transpose(x, w', stride=2, VALID)          # [129,129,256]
    y  = depthwise_FIR_4x4(pad(y1,1)) + b                # [128,128,256]

Implementation strategy (per NeuronCore, data-parallel over batch 16 -> 8
cores x 2 images; each image processed as 2 units of 128 output channels):

  Stage 1 (TensorE): subpixel decomposition of the stride-2 transpose conv.
    Output parity (rho,sig) of the upsampled grid is a stride-1 VALID conv
    of the zero-padded x with taps W[a,b], a = 2*di+rho.  Matmuls contract
    over in-channels (128 per chunk), fp32 data bitcast to float32r (FP22
    multiply, full PE rate at free-dim >= 256).  ScalarE interleaves the
    parity grids into a dense fp16 up-grid y1 (strided psum->SBUF copies).

  Stage 2: separable FIR as six 2-tap box passes ([1,3,3,1] = [1,1]^*3 per
    axis; the 1/16 normalisation is folded into W).  Three vertical passes
    (whole-row shifts) then three horizontal passes (1-col shifts), all
    fp16 tensor_tensor adds distributed between VectorE and GpSimd by a
    static load-balancing heuristic.  For the last unit the horizontal FIR
    runs on TensorE instead (4 accumulating diagonal matmuls per row
    group) -- the conv work is finished by then, so this shortens the tail.

  Output is written fp16 (channel-major [n, ocx, ch, r, s]); the host
  upcasts, transposes back to NHWC and adds the bias.
"""

import sys

sys.path.insert(0, "/opt/trn_rl_repo")

import numpy as np

import concourse.bass as bass  # noqa: F401  (registers engine classes)
import concourse.mybir as mybir
import concourse.tile as tile
from concourse import bacc
from concourse.bass_utils import run_bass_kernel_spmd

F32 = mybir.dt.float32
F32R = mybir.dt.float32r
F16 = mybir.dt.float16
ADD = mybir.AluOpType.add

N_CORES = 8
IMGS_PER_CORE = 2
H = W = 64          # input spatial
UP = 129            # upsampled grid (conv_transpose output)
OUT = 128           # final spatial
C = 256             # channels
CH = 128            # channels per partition chunk
BAND = 32           # FIR band rows (4 bands per unit)
GROUP = 4           # col-FIR psum group rows (4*128 = 512 free)

XROW = W + 2          # 66: padded x row length
XFLAT = (H + 2) * XROW  # 4356: flat padded image

# cost-model rates for the static DVE/GpSimd balance (ns per free elem,
# plus fixed per-instruction overhead)
_DVE_NS = 0.543
_DVE_FIX = 120.0
_POOL_NS = 1.984
_POOL_FIX = 160.0


def _build_nc():
    nc = bacc.Bacc("TRN2", target_bir_lowering=False)

    # x arrives host-padded to 66x66 (zero border) and channel-major
    # [n, icx, ch, h*w] so each partition's DMA run is contiguous
    x_d = nc.dram_tensor("x", [IMGS_PER_CORE, 2, CH, XFLAT], F32R, kind="ExternalInput")
    # Pre-arranged conv taps: [ic_part, icx, tap(a*3+b), ocx, oc]
    w_d = nc.dram_tensor("wt", [CH, 2, 9, 2, CH], F32R, kind="ExternalInput")
    # Diagonal FIR weights, fp16: [:,0:128] = I, [:,128:256] = 3I
    d_d = nc.dram_tensor("dg", [CH, 2 * CH], F16, kind="ExternalInput")
    # channel-major fp16 output [n, ocx, ch, r, s]; host transposes to NHWC
    y_d = nc.dram_tensor("y", [IMGS_PER_CORE, 2, CH, OUT, OUT], F16, kind="ExternalOutput")

    with tile.TileContext(nc) as tc:
        with (
            tc.tile_pool(name="const", bufs=1) as constp,
            tc.tile_pool(name="xp", bufs=1) as xp,
            tc.tile_pool(name="y1p", bufs=2) as y1p,
            tc.tile_pool(name="firp", bufs=2) as firp,
            tc.tile_pool(name="zp", bufs=2) as zp,
            tc.tile_pool(name="outp", bufs=3) as outp,
            tc.tile_pool(name="cpsum", bufs=4, space="PSUM") as cpsum,
            tc.tile_pool(name="fpsum", bufs=4, space="PSUM") as fpsum,
        ):
            w_sb = constp.tile([CH, 2, 9, 2, CH], F32R)
            nc.sync.dma_start(out=w_sb[:], in_=w_d[:])
            dg_sb = constp.tile([CH, 2 * CH], F16)
            nc.sync.dma_start(out=dg_sb[:], in_=d_d[:])

            # static engine balance state for the FIR box passes
            busy = {"d": 0.0, "p": 0.0}

            def boxadd(out, in0, in1, n_elems):
                cd = n_elems * _DVE_NS + _DVE_FIX
                cp = n_elems * _POOL_NS + _POOL_FIX
                if busy["d"] + cd <= busy["p"] + cp:
                    busy["d"] += cd
                    nc.vector.tensor_tensor(out=out, in0=in0, in1=in1, op=ADD)
                else:
                    busy["p"] += cp
                    nc.gpsimd.tensor_tensor(out=out, in0=in0, in1=in1, op=ADD)

            # flat x image + 2 slack elems so full-row matmul spans with a
            # column offset stay in bounds (fp32r matmuls need 2D-collapsible
            # APs, so the rhs is a contiguous span covering whole rows)
            x_sb = xp.tile([CH, 2, XFLAT + 2], F32R)
            nc.vector.memset(x_sb[:, 0, XFLAT : XFLAT + 2].bitcast(F32), 0.0)
            nc.vector.memset(x_sb[:, 1, XFLAT : XFLAT + 2].bitcast(F32), 0.0)

            for n in range(IMGS_PER_CORE):
                for icx in range(2):
                    nc.sync.dma_start(
                        out=x_sb[:, icx, 0:XFLAT],
                        in_=x_d[n, icx],
                    )
                for ocx in range(2):
                    unit = 2 * n + ocx
                    pe_h = unit == 3  # last unit: horizontal FIR on TensorE
                    # ---------------- stage 1: conv into y1 (fp16) ----------
                    # y1_sb rows: up-row p at index p+1 (rows 0,130,131 zero)
                    # cols: up-col q at index q (col 129 pad, never read)
                    y1_sb = y1p.tile([CH, UP + 3, UP + 1], F16, tag="y1")
                    nc.vector.memset(y1_sb[:, 0:1, 0:UP], 0.0)
                    nc.vector.memset(y1_sb[:, UP + 1 : UP + 3, 0:UP], 0.0)

                    for rho in range(2):
                        for sig in range(2):
                            nm, nn = 65 - rho, 65 - sig
                            dis = (0, 1) if rho == 0 else (0,)
                            djs = (0, 1) if sig == 0 else (0,)
                            m0 = 0
                            while m0 < nm:
                                r = min(7, nm - m0)
                                ps = cpsum.tile([CH, r, XROW], F32, tag="cps")
                                mms = [
                                    (di, dj, icx2)
                                    for di in dis
                                    for dj in djs
                                    for icx2 in range(2)
                                ]
                                for k, (di, dj, icx2) in enumerate(mms):
                                    t = (2 * di + rho) * 3 + (2 * dj + sig)
                                    st = (m0 + 1 - di) * XROW + (1 - dj)
                                    nc.tensor.matmul(
                                        ps[:, 0:r, 0:XROW].opt({0}),
                                        lhsT=w_sb[:, icx2, t, ocx, :],
                                        rhs=x_sb[:, icx2, st : st + r * XROW],
                                        start=(k == 0),
                                        stop=(k == len(mms) - 1),
                                    )
                                # strided parity write into the up-grid
                                # (cols nn..65 of each psum row are garbage
                                # from the full-row span and are skipped)
                                nc.scalar.copy(
                                    out=y1_sb[
                                        :,
                                        1 + rho + 2 * m0 : 1 + rho + 2 * (m0 + r) : 2,
                                        sig : sig + 2 * nn : 2,
                                    ],
                                    in_=ps[:, 0:r, 0:nn],
                                )
                                m0 += r

                    # ---------------- stage 2: FIR box passes per band ------
                    for r0 in range(0, OUT, BAND):
                        # vertical: z[r] = y1[r-1] + 3 y1[r] + 3 y1[r+1] + y1[r+2]
                        # (up-row p at y1 index p+1)
                        b1 = firp.tile([CH, BAND + 2, UP + 1], F16, tag="A")
                        boxadd(
                            b1[:, :, 0:UP],
                            y1_sb[:, r0 : r0 + BAND + 2, 0:UP],
                            y1_sb[:, r0 + 1 : r0 + BAND + 3, 0:UP],
                            (BAND + 2) * UP,
                        )
                        b2 = firp.tile([CH, BAND + 1, UP + 1], F16, tag="B")
                        boxadd(
                            b2[:, :, 0:UP],
                            b1[:, 0 : BAND + 1, 0:UP],
                            b1[:, 1 : BAND + 2, 0:UP],
                            (BAND + 1) * UP,
                        )
                        # z cols: up-col q at index q+2 (idx 1 and 131 zero)
                        z = zp.tile([CH, BAND, UP + 3], F16, tag="z")
                        nc.vector.memset(z[:, :, 1:2], 0.0)
                        nc.vector.memset(z[:, :, UP + 2 : UP + 3], 0.0)
                        boxadd(
                            z[:, :, 2 : UP + 2],
                            b2[:, 0:BAND, 0:UP],
                            b2[:, 1 : BAND + 1, 0:UP],
                            BAND * UP,
                        )

                        out_sb = outp.tile([CH, BAND, OUT], F16, tag="out")
                        if pe_h:
                            # horizontal FIR on TensorE: 4 accumulating
                            # diagonal matmuls per 4-row group
                            for g0 in range(0, BAND, GROUP):
                                ps2 = fpsum.tile([CH, GROUP, OUT], F32, tag="fps")
                                for v in range(4):
                                    dgi = 0 if v in (0, 3) else 1
                                    nc.tensor.matmul(
                                        ps2[:],
                                        lhsT=dg_sb[:, dgi * CH : (dgi + 1) * CH],
                                        rhs=z[:, g0 : g0 + GROUP, v + 1 : v + 1 + OUT],
                                        start=(v == 0),
                                        stop=(v == 3),
                                    )
                                nc.scalar.copy(
                                    out=out_sb[:, g0 : g0 + GROUP, :],
                                    in_=ps2[:],
                                )
                        else:
                            # horizontal FIR: three 1-col-shift box passes
                            h1 = firp.tile([CH, BAND, UP + 1], F16, tag="A")
                            boxadd(
                                h1[:, :, 0 : UP + 1],
                                z[:, :, 1 : UP + 2],
                                z[:, :, 2 : UP + 3],
                                BAND * (UP + 1),
                            )
                            h2 = firp.tile([CH, BAND, UP], F16, tag="B")
                            boxadd(
                                h2[:, :, 0:UP],
                                h1[:, :, 0:UP],
                                h1[:, :, 1 : UP + 1],
                                BAND * UP,
                            )
                            boxadd(
                                out_sb[:],
                                h2[:, :, 0:OUT],
                                h2[:, :, 1 : OUT + 1],
                                BAND * OUT,
                            )
                        nc.sync.dma_start(
                            out=y_d[n, ocx, :, r0 : r0 + BAND, :],
                            in_=out_sb[:],
                        )
    nc.compile()
    return nc


_NC_CACHE = None


def _get_nc():
    global _NC_CACHE
    if _NC_CACHE is None:
        _NC_CACHE = _build_nc()
    return _NC_CACHE


def kernel(x, w, b):
    x = np.asarray(x, dtype=np.float32)
    w = np.asarray(w, dtype=np.float32)
    b = np.asarray(b, dtype=np.float32)
    # channel-major + zero pad: [N, 2, CH, (H+2)*(W+2)]
    xt = np.zeros((x.shape[0], 2, CH, H + 2, W + 2), dtype=np.float32)
    xt[:, :, :, 1 : H + 1, 1 : W + 1] = x.transpose(0, 3, 1, 2).reshape(
        x.shape[0], 2, CH, H, W
    )
    xt = xt.reshape(x.shape[0], 2, CH, XFLAT)

    # Effective transpose-conv filter, with the separable FIR normalisation
    # (1/4 per axis) folded in.
    Wf = w[::-1, ::-1] / 16.0  # [a, b, ic, oc]
    Wr = Wf.reshape(3, 3, 2, CH, 2, CH)  # a, b, icx, ic, ocx, oc
    w_arr = np.ascontiguousarray(
        Wr.transpose(3, 2, 0, 1, 4, 5).reshape(CH, 2, 9, 2, CH)
    )
    eye = np.eye(CH, dtype=np.float16)
    dg = np.ascontiguousarray(np.concatenate([eye, 3.0 * eye], axis=1))

    in_maps = [
        {
            "x": np.ascontiguousarray(xt[IMGS_PER_CORE * c : IMGS_PER_CORE * (c + 1)]),
            "wt": w_arr,
            "dg": dg,
        }
        for c in range(N_CORES)
    ]
    nc = _get_nc()
    res = run_bass_kernel_spmd(nc, in_maps, core_ids=list(range(N_CORES)))
    # [n, 2, CH, r, s] fp16 -> [n, r, s, 2*CH] fp32 + bias
    y = np.concatenate([res.results[c]["y"] for c in range(N_CORES)], axis=0)
    y = y.reshape(-1, C, OUT, OUT).transpose(0, 2, 3, 1).astype(np.float32)
    y += b.reshape(1, 1, 1, C)
    return np.ascontiguousarray(y)


if __name__ == "__main__":
    rng = np.random.default_rng(0)
    x = rng.standard_normal((16, 64, 64, 256), dtype=np.float32)
    w = rng.standard_normal((3, 3, 256, 256), dtype=np.float32) * 0.02
    b = np.zeros((256,), dtype=np.float32)
    y = kernel(x, w, b)
    print("out:", y.shape, y.dtype)


# revision 6
# speedup vs baseline: 1.0866x; 1.0866x over previous
"""Trainium2 Bass kernel for StyleGAN2-style fused upsample(x2)+conv3x3+FIR.

Reference computation (per image):
    y1 = conv_transpose(x, w', stride=2, VALID)          # [129,129,256]
    y  = depthwise_FIR_4x4(pad(y1,1)) + b                # [128,128,256]

Implementation strategy (per NeuronCore, data-parallel over batch 16 -> 8
cores x 2 images; each image processed as 2 units of 128 output channels):

  Stage 1 (TensorE): subpixel decomposition of the stride-2 transpose conv.
    Output parity (rho,sig) of the upsampled grid is a stride-1 VALID conv
    of the zero-padded x with taps W[a,b], a = 2*di+rho.  Matmuls contract
    over in-channels (128 per chunk), fp32 data bitcast to float32r (FP22
    multiply, full PE rate at free-dim >= 256).  ScalarE interleaves the
    parity grids into a dense fp16 up-grid y1 (strided psum->SBUF copies).

  Stage 2: separable FIR as six 2-tap box passes ([1,3,3,1] = [1,1]^*3 per
    axis; the 1/16 normalisation is folded into W).  Three vertical passes
    (whole-row shifts) then three horizontal passes (1-col shifts), all
    fp16 tensor_tensor adds distributed between VectorE and GpSimd by a
    static load-balancing heuristic.  For the last unit the horizontal FIR
    runs on TensorE instead (4 accumulating diagonal matmuls per row
    group) -- the conv work is finished by then, so this shortens the tail.

  Output is written fp16 (channel-major [n, ocx, ch, r, s]); the host
  upcasts, transposes back to NHWC and adds the bias.
"""

import sys

sys.path.insert(0, "/opt/trn_rl_repo")

import numpy as np

import concourse.bass as bass  # noqa: F401  (registers engine classes)
import concourse.mybir as mybir
import concourse.tile as tile
from concourse import bacc
from concourse.bass_utils import run_bass_kernel_spmd

F32 = mybir.dt.float32
F32R = mybir.dt.float32r
F16 = mybir.dt.float16
ADD = mybir.AluOpType.add

N_CORES = 8
IMGS_PER_CORE = 2
H = W = 64          # input spatial
UP = 129            # upsampled grid (conv_transpose output)
OUT = 128           # final spatial
C = 256             # channels
CH = 128            # channels per partition chunk
BAND = 32           # FIR band rows (4 bands per unit)
GROUP = 4           # col-FIR psum group rows (4*128 = 512 free)

XROW = W + 2          # 66: padded x row length
XFLAT = (H + 2) * XROW  # 4356: flat padded image

# cost-model rates for the static DVE/GpSimd balance (ns per free elem,
# plus fixed per-instruction overhead)
_DVE_NS = 0.543
_DVE_FIX = 120.0
_POOL_NS = 1.984
_POOL_FIX = 160.0


def _build_nc():
    nc = bacc.Bacc("TRN2", target_bir_lowering=False)

    # x arrives host-padded to 66x66 (zero border) and channel-major
    # [n, icx, ch, h*w] so each partition's DMA run is contiguous
    x_d = nc.dram_tensor("x", [IMGS_PER_CORE, 2, CH, XFLAT], F32R, kind="ExternalInput")
    # Pre-arranged conv taps: [ic_part, icx, tap(a*3+b), ocx, oc]
    w_d = nc.dram_tensor("wt", [CH, 2, 9, 2, CH], F32R, kind="ExternalInput")
    # Diagonal FIR weights, fp16: [:,0:128] = I, [:,128:256] = 3I
    d_d = nc.dram_tensor("dg", [CH, 2 * CH], F16, kind="ExternalInput")
    # channel-major fp16 output [n, ocx, ch, r, s]; host transposes to NHWC
    y_d = nc.dram_tensor("y", [IMGS_PER_CORE, 2, CH, OUT, OUT], F16, kind="ExternalOutput")

    with tile.TileContext(nc) as tc:
        with (
            tc.tile_pool(name="const", bufs=1) as constp,
            tc.tile_pool(name="xp", bufs=1) as xp,
            tc.tile_pool(name="y1p", bufs=2) as y1p,
            tc.tile_pool(name="firp", bufs=2) as firp,
            tc.tile_pool(name="zp", bufs=2) as zp,
            tc.tile_pool(name="outp", bufs=3) as outp,
            tc.tile_pool(name="cpsum", bufs=4, space="PSUM") as cpsum,
            tc.tile_pool(name="fpsum", bufs=4, space="PSUM") as fpsum,
        ):
            w_sb = constp.tile([CH, 2, 9, 2, CH], F32R)
            # ocx=0 taps first so unit 0's matmuls can start sooner
            nc.sync.dma_start(out=w_sb[:, :, :, 0, :], in_=w_d[:, :, :, 0, :])

            # static engine balance state for the FIR box passes
            busy = {"d": 0.0, "p": 0.0}

            def boxadd(out, in0, in1, n_elems, allow_pool=True):
                cd = n_elems * _DVE_NS + _DVE_FIX
                cp = n_elems * _POOL_NS + _POOL_FIX
                if not allow_pool or busy["d"] + cd <= busy["p"] + cp:
                    busy["d"] += cd
                    nc.vector.tensor_tensor(out=out, in0=in0, in1=in1, op=ADD)
                else:
                    busy["p"] += cp
                    nc.gpsimd.tensor_tensor(out=out, in0=in0, in1=in1, op=ADD)

            # flat x image + 2 slack elems so full-row matmul spans with a
            # column offset stay in bounds (fp32r matmuls need 2D-collapsible
            # APs, so the rhs is a contiguous span covering whole rows)
            x_sb = xp.tile([CH, 2, XFLAT + 2], F32R)
            nc.vector.memset(x_sb[:, 0, XFLAT : XFLAT + 2].bitcast(F32), 0.0)
            nc.vector.memset(x_sb[:, 1, XFLAT : XFLAT + 2].bitcast(F32), 0.0)

            first_dma = True
            for n in range(IMGS_PER_CORE):
                for icx in range(2):
                    nc.sync.dma_start(
                        out=x_sb[:, icx, 0:XFLAT],
                        in_=x_d[n, icx],
                    )
                if first_dma:
                    # remaining constants after unit 0's critical inputs
                    nc.sync.dma_start(
                        out=w_sb[:, :, :, 1, :], in_=w_d[:, :, :, 1, :]
                    )
                    dg_sb = constp.tile([CH, 2 * CH], F16)
                    nc.sync.dma_start(out=dg_sb[:], in_=d_d[:])
                    first_dma = False
                for ocx in range(2):
                    unit = 2 * n + ocx
                    pe_h = unit == 3  # last unit: horizontal FIR on TensorE
                    # ---------------- stage 1: conv into y1 (fp16) ----------
                    # y1_sb rows: up-row p at index p+1 (rows 0,130,131 zero)
                    # cols: up-col q at index q (col 129 pad, never read)
                    y1_sb = y1p.tile([CH, UP + 3, UP + 1], F16, tag="y1")
                    nc.vector.memset(y1_sb[:, 0:1, 0:UP], 0.0)
                    nc.vector.memset(y1_sb[:, UP + 1 : UP + 3, 0:UP], 0.0)

                    # band-major over up-row chunks: all 4 parities per
                    # m-chunk so the FIR bands can start while later rows
                    # are still being computed
                    for m0 in range(0, 65, 7):
                        for rho in range(2):
                            for sig in range(2):
                                nm, nn = 65 - rho, 65 - sig
                                if m0 >= nm:
                                    continue
                                dis = (0, 1) if rho == 0 else (0,)
                                djs = (0, 1) if sig == 0 else (0,)
                                r = min(7, nm - m0)
                                ps = cpsum.tile([CH, r, XROW], F32, tag="cps")
                                # icx-major accumulation order: the first
                                # matmuls only need x[icx=0], overlapping
                                # with the x[icx=1] DMA on unit 0
                                mms = [
                                    (di, dj, icx2)
                                    for icx2 in range(2)
                                    for di in dis
                                    for dj in djs
                                ]
                                for k, (di, dj, icx2) in enumerate(mms):
                                    t = (2 * di + rho) * 3 + (2 * dj + sig)
                                    st = (m0 + 1 - di) * XROW + (1 - dj)
                                    nc.tensor.matmul(
                                        ps[:, 0:r, 0:XROW].opt({0}),
                                        lhsT=w_sb[:, icx2, t, ocx, :],
                                        rhs=x_sb[:, icx2, st : st + r * XROW],
                                        start=(k == 0),
                                        stop=(k == len(mms) - 1),
                                    )
                                # strided parity write into the up-grid
                                # (cols nn..65 of each psum row are garbage
                                # from the full-row span and are skipped)
                                nc.scalar.copy(
                                    out=y1_sb[
                                        :,
                                        1 + rho + 2 * m0 : 1 + rho + 2 * (m0 + r) : 2,
                                        sig : sig + 2 * nn : 2,
                                    ],
                                    in_=ps[:, 0:r, 0:nn],
                                )

                    # ---------------- stage 2: FIR box passes per band ------
                    for r0 in range(0, OUT, BAND):
                        # vertical: z[r] = y1[r-1] + 3 y1[r] + 3 y1[r+1] + y1[r+2]
                        # (up-row p at y1 index p+1)
                        b1 = firp.tile([CH, BAND + 2, UP + 1], F16, tag="A")
                        boxadd(
                            b1[:, :, 0:UP],
                            y1_sb[:, r0 : r0 + BAND + 2, 0:UP],
                            y1_sb[:, r0 + 1 : r0 + BAND + 3, 0:UP],
                            (BAND + 2) * UP,
                            allow_pool=not pe_h,
                        )
                        b2 = firp.tile([CH, BAND + 1, UP + 1], F16, tag="B")
                        boxadd(
                            b2[:, :, 0:UP],
                            b1[:, 0 : BAND + 1, 0:UP],
                            b1[:, 1 : BAND + 2, 0:UP],
                            (BAND + 1) * UP,
                            allow_pool=not pe_h,
                        )
                        # z cols: up-col q at index q+2 (idx 1 and 131 zero)
                        z = zp.tile([CH, BAND, UP + 3], F16, tag="z")
                        nc.vector.memset(z[:, :, 1:2], 0.0)
                        nc.vector.memset(z[:, :, UP + 2 : UP + 3], 0.0)
                        boxadd(
                            z[:, :, 2 : UP + 2],
                            b2[:, 0:BAND, 0:UP],
                            b2[:, 1 : BAND + 1, 0:UP],
                            BAND * UP,
                            allow_pool=not pe_h,
                        )

                        out_sb = outp.tile([CH, BAND, OUT], F16, tag="out")
                        if pe_h:
                            # horizontal FIR on TensorE: 4 accumulating
                            # diagonal matmuls per 4-row group
                            for g0 in range(0, BAND, GROUP):
                                ps2 = fpsum.tile([CH, GROUP, OUT], F32, tag="fps")
                                for v in range(4):
                                    dgi = 0 if v in (0, 3) else 1
                                    nc.tensor.matmul(
                                        ps2[:],
                                        lhsT=dg_sb[:, dgi * CH : (dgi + 1) * CH],
                                        rhs=z[:, g0 : g0 + GROUP, v + 1 : v + 1 + OUT],
                                        start=(v == 0),
                                        stop=(v == 3),
                                    )
                                nc.scalar.copy(
                                    out=out_sb[:, g0 : g0 + GROUP, :],
                                    in_=ps2[:],
                                )
                        else:
                            # horizontal FIR: three 1-col-shift box passes
                            h1 = firp.tile([CH, BAND, UP + 1], F16, tag="A")
                            boxadd(
                                h1[:, :, 0 : UP + 1],
                                z[:, :, 1 : UP + 2],
                                z[:, :, 2 : UP + 3],
                                BAND * (UP + 1),
                            )
                            h2 = firp.tile([CH, BAND, UP], F16, tag="B")
                            boxadd(
                                h2[:, :, 0:UP],
                                h1[:, :, 0:UP],
                                h1[:, :, 1 : UP + 1],
                                BAND * UP,
                            )
                            boxadd(
                                out_sb[:],
                                h2[:, :, 0:OUT],
                                h2[:, :, 1 : OUT + 1],
                                BAND * OUT,
                            )
                        nc.sync.dma_start(
                            out=y_d[n, ocx, :, r0 : r0 + BAND, :],
                            in_=out_sb[:],
                        )
    nc.compile()
    return nc


_NC_CACHE = None


def _get_nc():
    global _NC_CACHE
    if _NC_CACHE is None:
        _NC_CACHE = _build_nc()
    return _NC_CACHE


def kernel(x, w, b):
    x = np.asarray(x, dtype=np.float32)
    w = np.asarray(w, dtype=np.float32)
    b = np.asarray(b, dtype=np.float32)
    # channel-major + zero pad: [N, 2, CH, (H+2)*(W+2)]
    xt = np.zeros((x.shape[0], 2, CH, H + 2, W + 2), dtype=np.float32)
    xt[:, :, :, 1 : H + 1, 1 : W + 1] = x.transpose(0, 3, 1, 2).reshape(
        x.shape[0], 2, CH, H, W
    )
    xt = xt.reshape(x.shape[0], 2, CH, XFLAT)

    # Effective transpose-conv filter, with the separable FIR normalisation
    # (1/4 per axis) folded in.
    Wf = w[::-1, ::-1] / 16.0  # [a, b, ic, oc]
    Wr = Wf.reshape(3, 3, 2, CH, 2, CH)  # a, b, icx, ic, ocx, oc
    w_arr = np.ascontiguousarray(
        Wr.transpose(3, 2, 0, 1, 4, 5).reshape(CH, 2, 9, 2, CH)
    )
    eye = np.eye(CH, dtype=np.float16)
    dg = np.ascontiguousarray(np.concatenate([eye, 3.0 * eye], axis=1))

    in_maps = [
        {
            "x": np.ascontiguousarray(xt[IMGS_PER_CORE * c : IMGS_PER_CORE * (c + 1)]),
            "wt": w_arr,
            "dg": dg,
        }
        for c in range(N_CORES)
    ]
    nc = _get_nc()
    res = run_bass_kernel_spmd(nc, in_maps, core_ids=list(range(N_CORES)))
    # [n, 2, CH, r, s] fp16 -> [n, r, s, 2*CH] fp32 + bias
    y = np.concatenate([res.results[c]["y"] for c in range(N_CORES)], axis=0)
    y = y.reshape(-1, C, OUT, OUT).transpose(0, 2, 3, 1).astype(np.float32)
    y += b.reshape(1, 1, 1, C)
    return np.ascontiguousarray(y)


if __name__ == "__main__":
    rng = np.random.default_rng(0)
    x = rng.standard_normal((16, 64, 64, 256), dtype=np.float32)
    w = rng.standard_normal((3, 3, 256, 256), dtype=np.float32) * 0.02
    b = np.zeros((256,), dtype=np.float32)
    y = kernel(x, w, b)
    print("out:", y.shape, y.dtype)


# revision 7
# speedup vs baseline: 1.0866x; 1.0000x over previous
"""Trainium2 Bass kernel for StyleGAN2-style fused upsample(x2)+conv3x3+FIR.

Reference computation (per image):
    y1 = conv_transpose(x, w', stride=2, VALID)          # [129,129,256]
    y  = depthwise_FIR_4x4(pad(y1,1)) + b                # [128,128,256]

Implementation strategy (per NeuronCore, data-parallel over batch 16 -> 8
cores x 2 images; each image processed as 2 units of 128 output channels):

  Stage 1 (TensorE): subpixel decomposition of the stride-2 transpose conv.
    Output parity (rho,sig) of the upsampled grid is a stride-1 VALID conv
    of the zero-padded x with taps W[a,b], a = 2*di+rho.  Matmuls contract
    over in-channels (128 per chunk), fp32 data bitcast to float32r (FP22
    multiply, full PE rate at free-dim >= 256).  ScalarE interleaves the
    parity grids into a dense fp16 up-grid y1 (strided psum->SBUF copies).

  Stage 2: separable FIR as six 2-tap box passes ([1,3,3,1] = [1,1]^*3 per
    axis; the 1/16 normalisation is folded into W).  Three vertical passes
    (whole-row shifts) then three horizontal passes (1-col shifts), all
    fp16 tensor_tensor adds distributed between VectorE and GpSimd by a
    static load-balancing heuristic.  For the last unit the horizontal FIR
    runs on TensorE instead (4 accumulating diagonal matmuls per row
    group) -- the conv work is finished by then, so this shortens the tail.

  Output is written fp16 (channel-major [n, ocx, ch, r, s]); the host
  upcasts, transposes back to NHWC and adds the bias.
"""

import sys

sys.path.insert(0, "/opt/trn_rl_repo")

import numpy as np

import concourse.bass as bass  # noqa: F401  (registers engine classes)
import concourse.mybir as mybir
import concourse.tile as tile
from concourse import bacc
from concourse.bass_utils import run_bass_kernel_spmd

F32 = mybir.dt.float32
F32R = mybir.dt.float32r
F16 = mybir.dt.float16
ADD = mybir.AluOpType.add

N_CORES = 8
IMGS_PER_CORE = 2
H = W = 64          # input spatial
UP = 129            # upsampled grid (conv_transpose output)
OUT = 128           # final spatial
C = 256             # channels
CH = 128            # channels per partition chunk
BAND = 16           # FIR band rows (8 bands per unit)
GROUP = 4           # col-FIR psum group rows (4*128 = 512 free)

XROW = W + 2          # 66: padded x row length
XFLAT = (H + 2) * XROW  # 4356: flat padded image

# cost-model rates for the static DVE/GpSimd balance (ns per free elem,
# plus fixed per-instruction overhead)
_DVE_NS = 0.543
_DVE_FIX = 120.0
_POOL_NS = 1.984
_POOL_FIX = 160.0


def _build_nc():
    nc = bacc.Bacc("TRN2", target_bir_lowering=False)

    # x arrives host-padded to 66x66 (zero border) and channel-major
    # [n, icx, ch, h*w] so each partition's DMA run is contiguous
    x_d = nc.dram_tensor("x", [IMGS_PER_CORE, 2, CH, XFLAT], F32R, kind="ExternalInput")
    # Pre-arranged conv taps: [ic_part, icx, tap(a*3+b), ocx, oc]
    w_d = nc.dram_tensor("wt", [CH, 2, 9, 2, CH], F32R, kind="ExternalInput")
    # Diagonal FIR weights, fp16: [:,0:128] = I, [:,128:256] = 3I
    d_d = nc.dram_tensor("dg", [CH, 2 * CH], F16, kind="ExternalInput")
    # channel-major fp16 output [n, ocx, ch, r, s]; host transposes to NHWC
    y_d = nc.dram_tensor("y", [IMGS_PER_CORE, 2, CH, OUT, OUT], F16, kind="ExternalOutput")

    with tile.TileContext(nc) as tc:
        with (
            tc.tile_pool(name="const", bufs=1) as constp,
            tc.tile_pool(name="xp", bufs=2) as xp,
            tc.tile_pool(name="y1p", bufs=1) as y1p,
            tc.tile_pool(name="firp", bufs=4) as firp,
            tc.tile_pool(name="zp", bufs=4) as zp,
            tc.tile_pool(name="outp", bufs=4) as outp,
            tc.tile_pool(name="cpsum", bufs=4, space="PSUM") as cpsum,
            tc.tile_pool(name="fpsum", bufs=4, space="PSUM") as fpsum,
        ):
            w_sb = constp.tile([CH, 2, 9, 2, CH], F32R)
            # ocx=0 taps first so unit 0's matmuls can start sooner
            nc.sync.dma_start(out=w_sb[:, :, :, 0, :], in_=w_d[:, :, :, 0, :])

            # static engine balance state for the FIR box passes
            busy = {"d": 0.0, "p": 0.0}

            def boxadd(out, in0, in1, n_elems, allow_pool=True):
                cd = n_elems * _DVE_NS + _DVE_FIX
                cp = n_elems * _POOL_NS + _POOL_FIX
                if not allow_pool or busy["d"] + cd <= busy["p"] + cp:
                    busy["d"] += cd
                    nc.vector.tensor_tensor(out=out, in0=in0, in1=in1, op=ADD)
                else:
                    busy["p"] += cp
                    nc.gpsimd.tensor_tensor(out=out, in0=in0, in1=in1, op=ADD)

            first_dma = True
            for n in range(IMGS_PER_CORE):
                # flat x image + 2 slack elems so full-row matmul spans with
                # a column offset stay in bounds (fp32r matmuls need
                # 2D-collapsible APs: contiguous spans covering whole rows);
                # double-buffered so image 1's DMA overlaps image 0's conv
                x_sb = xp.tile([CH, 2, XFLAT + 2], F32R, tag="x")
                nc.vector.memset(x_sb[:, 0, XFLAT : XFLAT + 2].bitcast(F32), 0.0)
                nc.vector.memset(x_sb[:, 1, XFLAT : XFLAT + 2].bitcast(F32), 0.0)
                for icx in range(2):
                    nc.sync.dma_start(
                        out=x_sb[:, icx, 0:XFLAT],
                        in_=x_d[n, icx],
                    )
                if first_dma:
                    # remaining constants after unit 0's critical inputs
                    nc.sync.dma_start(
                        out=w_sb[:, :, :, 1, :], in_=w_d[:, :, :, 1, :]
                    )
                    dg_sb = constp.tile([CH, 2 * CH], F16)
                    nc.sync.dma_start(out=dg_sb[:], in_=d_d[:])
                    first_dma = False
                for ocx in range(2):
                    unit = 2 * n + ocx
                    pe_h = unit == 3  # last unit: horizontal FIR on TensorE
                    # ---------------- stage 1: conv into y1 (fp16) ----------
                    # y1_sb rows: up-row p at index p+1 (rows 0,130,131 zero)
                    # cols: up-col q at index q (col 129 pad, never read)
                    y1_sb = y1p.tile([CH, UP + 3, UP + 1], F16, tag="y1")
                    nc.vector.memset(y1_sb[:, 0:1, 0:UP], 0.0)
                    nc.vector.memset(y1_sb[:, UP + 1 : UP + 3, 0:UP], 0.0)

                    # band-major over up-row chunks: all 4 parities per
                    # m-chunk so the FIR bands can start while later rows
                    # are still being computed
                    for m0 in range(0, 65, 7):
                        for rho in range(2):
                            for sig in range(2):
                                nm, nn = 65 - rho, 65 - sig
                                if m0 >= nm:
                                    continue
                                dis = (0, 1) if rho == 0 else (0,)
                                djs = (0, 1) if sig == 0 else (0,)
                                r = min(7, nm - m0)
                                ps = cpsum.tile([CH, r, XROW], F32, tag="cps")
                                # icx-major accumulation order: the first
                                # matmuls only need x[icx=0], overlapping
                                # with the x[icx=1] DMA on unit 0
                                mms = [
                                    (di, dj, icx2)
                                    for icx2 in range(2)
                                    for di in dis
                                    for dj in djs
                                ]
                                for k, (di, dj, icx2) in enumerate(mms):
                                    t = (2 * di + rho) * 3 + (2 * dj + sig)
                                    st = (m0 + 1 - di) * XROW + (1 - dj)
                                    nc.tensor.matmul(
                                        ps[:, 0:r, 0:XROW].opt({0}),
                                        lhsT=w_sb[:, icx2, t, ocx, :],
                                        rhs=x_sb[:, icx2, st : st + r * XROW],
                                        start=(k == 0),
                                        stop=(k == len(mms) - 1),
                                    )
                                # strided parity write into the up-grid
                                # (cols nn..65 of each psum row are garbage
                                # from the full-row span and are skipped)
                                nc.scalar.copy(
                                    out=y1_sb[
                                        :,
                                        1 + rho + 2 * m0 : 1 + rho + 2 * (m0 + r) : 2,
                                        sig : sig + 2 * nn : 2,
                                    ],
                                    in_=ps[:, 0:r, 0:nn],
                                )

                    # ---------------- stage 2: FIR box passes per band ------
                    for r0 in range(0, OUT, BAND):
                        # vertical: z[r] = y1[r-1] + 3 y1[r] + 3 y1[r+1] + y1[r+2]
                        # (up-row p at y1 index p+1)
                        b1 = firp.tile([CH, BAND + 2, UP + 1], F16, tag="A")
                        boxadd(
                            b1[:, :, 0:UP],
                            y1_sb[:, r0 : r0 + BAND + 2, 0:UP],
                            y1_sb[:, r0 + 1 : r0 + BAND + 3, 0:UP],
                            (BAND + 2) * UP,
                            allow_pool=not pe_h,
                        )
                        b2 = firp.tile([CH, BAND + 1, UP + 1], F16, tag="B")
                        boxadd(
                            b2[:, :, 0:UP],
                            b1[:, 0 : BAND + 1, 0:UP],
                            b1[:, 1 : BAND + 2, 0:UP],
                            (BAND + 1) * UP,
                            allow_pool=not pe_h,
                        )
                        # z cols: up-col q at index q+2 (idx 1 and 131 zero)
                        z = zp.tile([CH, BAND, UP + 3], F16, tag="z")
                        nc.vector.memset(z[:, :, 1:2], 0.0)
                        nc.vector.memset(z[:, :, UP + 2 : UP + 3], 0.0)
                        boxadd(
                            z[:, :, 2 : UP + 2],
                            b2[:, 0:BAND, 0:UP],
                            b2[:, 1 : BAND + 1, 0:UP],
                            BAND * UP,
                            allow_pool=not pe_h,
                        )

                        out_sb = outp.tile([CH, BAND, OUT], F16, tag="out")
                        if pe_h:
                            # horizontal FIR on TensorE: 4 accumulating
                            # diagonal matmuls per 4-row group
                            for g0 in range(0, BAND, GROUP):
                                ps2 = fpsum.tile([CH, GROUP, OUT], F32, tag="fps")
                                for v in range(4):
                                    dgi = 0 if v in (0, 3) else 1
                                    nc.tensor.matmul(
                                        ps2[:],
                                        lhsT=dg_sb[:, dgi * CH : (dgi + 1) * CH],
                                        rhs=z[:, g0 : g0 + GROUP, v + 1 : v + 1 + OUT],
                                        start=(v == 0),
                                        stop=(v == 3),
                                    )
                                nc.scalar.copy(
                                    out=out_sb[:, g0 : g0 + GROUP, :],
                                    in_=ps2[:],
                                )
                        else:
                            # horizontal FIR: three 1-col-shift box passes
                            h1 = firp.tile([CH, BAND, UP + 1], F16, tag="A")
                            boxadd(
                                h1[:, :, 0 : UP + 1],
                                z[:, :, 1 : UP + 2],
                                z[:, :, 2 : UP + 3],
                                BAND * (UP + 1),
                            )
                            h2 = firp.tile([CH, BAND, UP], F16, tag="B")
                            boxadd(
                                h2[:, :, 0:UP],
                                h1[:, :, 0:UP],
                                h1[:, :, 1 : UP + 1],
                                BAND * UP,
                            )
                            boxadd(
                                out_sb[:],
                                h2[:, :, 0:OUT],
                                h2[:, :, 1 : OUT + 1],
                                BAND * OUT,
                            )
                        nc.sync.dma_start(
                            out=y_d[n, ocx, :, r0 : r0 + BAND, :],
                            in_=out_sb[:],
                        )
    nc.compile()
    return nc


_NC_CACHE = None


def _get_nc():
    global _NC_CACHE
    if _NC_CACHE is None:
        _NC_CACHE = _build_nc()
    return _NC_CACHE


def kernel(x, w, b):
    x = np.asarray(x, dtype=np.float32)
    w = np.asarray(w, dtype=np.float32)
    b = np.asarray(b, dtype=np.float32)
    # channel-major + zero pad: [N, 2, CH, (H+2)*(W+2)]
    xt = np.zeros((x.shape[0], 2, CH, H + 2, W + 2), dtype=np.float32)
    xt[:, :, :, 1 : H + 1, 1 : W + 1] = x.transpose(0, 3, 1, 2).reshape(
        x.shape[0], 2, CH, H, W
    )
    xt = xt.reshape(x.shape[0], 2, CH, XFLAT)

    # Effective transpose-conv filter, with the separable FIR normalisation
    # (1/4 per axis) folded in.
    Wf = w[::-1, ::-1] / 16.0  # [a, b, ic, oc]
    Wr = Wf.reshape(3, 3, 2, CH, 2, CH)  # a, b, icx, ic, ocx, oc
    w_arr = np.ascontiguousarray(
        Wr.transpose(3, 2, 0, 1, 4, 5).reshape(CH, 2, 9, 2, CH)
    )
    eye = np.eye(CH, dtype=np.float16)
    dg = np.ascontiguousarray(np.concatenate([eye, 3.0 * eye], axis=1))

    in_maps = [
        {
            "x": np.ascontiguousarray(xt[IMGS_PER_CORE * c : IMGS_PER_CORE * (c + 1)]),
            "wt": w_arr,
            "dg": dg,
        }
        for c in range(N_CORES)
    ]
    nc = _get_nc()
    res = run_bass_kernel_spmd(nc, in_maps, core_ids=list(range(N_CORES)))
    # [n, 2, CH, r, s] fp16 -> [n, r, s, 2*CH] fp32 + bias
    y = np.concatenate([res.results[c]["y"] for c in range(N_CORES)], axis=0)
    y = y.reshape(-1, C, OUT, OUT).transpose(0, 2, 3, 1).astype(np.float32)
    y += b.reshape(1, 1, 1, C)
    return np.ascontiguousarray(y)


if __name__ == "__main__":
    rng = np.random.default_rng(0)
    x = rng.standard_normal((16, 64, 64, 256), dtype=np.float32)
    w = rng.standard_normal((3, 3, 256, 256), dtype=np.float32) * 0.02
    b = np.zeros((256,), dtype=np.float32)
    y = kernel(x, w, b)
    print("out:", y.shape, y.dtype)


# revision 12
# speedup vs baseline: 1.1906x; 1.0957x over previous
"""Trainium2 Bass kernel for StyleGAN2-style fused upsample(x2)+conv3x3+FIR.

Reference computation (per image):
    y1 = conv_transpose(x, w', stride=2, VALID)          # [129,129,256]
    y  = depthwise_FIR_4x4(pad(y1,1)) + b                # [128,128,256]

Implementation strategy (per NeuronCore, data-parallel over batch 16 -> 8
cores x 2 images; each image processed as 2 units of 128 output channels):

  Stage 1 (TensorE): subpixel decomposition of the stride-2 transpose conv.
    Output parity (rho,sig) of the upsampled grid is a stride-1 VALID conv
    of the zero-padded x with taps W[a,b], a = 2*di+rho.  Matmuls contract
    over in-channels (128 per chunk), fp32 data bitcast to float32r (FP22
    multiply, full PE rate at free-dim >= 256).  ScalarE interleaves the
    parity grids into a dense fp16 up-grid y1 (strided psum->SBUF copies).

  Stage 2: separable FIR as six 2-tap box passes ([1,3,3,1] = [1,1]^*3 per
    axis; the 1/16 normalisation is folded into W).  Three vertical passes
    (whole-row shifts) then three horizontal passes (1-col shifts), all
    fp16 tensor_tensor adds distributed between VectorE and GpSimd by a
    static load-balancing heuristic.  For the last unit the horizontal FIR
    runs on TensorE instead (4 accumulating diagonal matmuls per row
    group) -- the conv work is finished by then, so this shortens the tail.

  Output is written fp16 (channel-major [n, ocx, ch, r, s]); the host
  upcasts, transposes back to NHWC and adds the bias.
"""

import sys

sys.path.insert(0, "/opt/trn_rl_repo")

import numpy as np

import concourse.bass as bass  # noqa: F401  (registers engine classes)
import concourse.mybir as mybir
import concourse.tile as tile
from concourse import bacc
from concourse.bass_utils import run_bass_kernel_spmd

F32 = mybir.dt.float32
F32R = mybir.dt.float32r
F16 = mybir.dt.float16
ADD = mybir.AluOpType.add

N_CORES = 8
IMGS_PER_CORE = 2
H = W = 64          # input spatial
UP = 129            # upsampled grid (conv_transpose output)
OUT = 128           # final spatial
C = 256             # channels
CH = 128            # channels per partition chunk
BAND = 16           # FIR band rows (8 bands per unit)
GROUP = 4           # col-FIR psum group rows (4*128 = 512 free)

XROW = W + 2          # 66: padded x row length
XFLAT = (H + 2) * XROW  # 4356: flat padded image

# cost-model rates for the static DVE/GpSimd balance (ns per free elem,
# plus fixed per-instruction overhead)
_DVE_NS = 0.543
_DVE_FIX = 120.0
_POOL_NS = 1.984
_POOL_FIX = 160.0

# (unit, band) pairs whose horizontal FIR runs on TensorE.  Emission of the
# matmuls is deferred into the next unit's conv chunks (the z tile is ready
# by then), so the in-order PE queue never stalls; the last unit's PEH bands
# are emitted at the very end, where PE is otherwise idle.
PEH_BANDS = {
    (0, 5), (0, 6), (0, 7),
    (1, 5), (1, 6), (1, 7),
    (2, 5), (2, 6), (2, 7),
    (3, 3), (3, 4), (3, 5), (3, 6), (3, 7),
}
# first conv m-chunk (of 10) at which deferred PEH bands may be flushed
_FLUSH_START = 6
# (unit, band) whose box passes must stay off GpSimd (schedule tail)
_POOL_DENY = {(3, b) for b in range(2, 8)}


def _build_nc():
    nc = bacc.Bacc("TRN2", target_bir_lowering=False)

    # x arrives host-padded to 66x66 (zero border) and channel-major
    # [n, icx, ch, h*w] so each partition's DMA run is contiguous
    x_d = nc.dram_tensor("x", [IMGS_PER_CORE, 2, CH, XFLAT], F32R, kind="ExternalInput")
    # Pre-arranged conv taps: [ic_part, icx, tap(a*3+b), ocx, oc]
    w_d = nc.dram_tensor("wt", [CH, 2, 9, 2, CH], F32R, kind="ExternalInput")
    # Diagonal FIR weights, fp16: [:,0:128] = I, [:,128:256] = 3I
    d_d = nc.dram_tensor("dg", [CH, 2 * CH], F16, kind="ExternalInput")
    # channel-major fp16 output [n, ocx, ch, r, s]; host transposes to NHWC
    y_d = nc.dram_tensor("y", [IMGS_PER_CORE, 2, CH, OUT, OUT], F16, kind="ExternalOutput")

    with tile.TileContext(nc) as tc:
        with (
            tc.tile_pool(name="const", bufs=1) as constp,
            tc.tile_pool(name="xp", bufs=2) as xp,
            tc.tile_pool(name="y1p", bufs=1) as y1p,
            tc.tile_pool(name="firp", bufs=3) as firp,
            tc.tile_pool(name="zp", bufs=4) as zp,
            tc.tile_pool(name="zdp", bufs=5) as zdp,
            tc.tile_pool(name="outp", bufs=3) as outp,
            tc.tile_pool(name="cpsum", bufs=4, space="PSUM") as cpsum,
            tc.tile_pool(name="fpsum", bufs=4, space="PSUM") as fpsum,
        ):
            w_sb = constp.tile([CH, 2, 9, 2, CH], F32R)
            # ocx=0 taps first so unit 0's matmuls can start sooner
            nc.sync.dma_start(out=w_sb[:, :, :, 0, :], in_=w_d[:, :, :, 0, :])

            # static engine balance state for the FIR box passes
            busy = {"d": 0.0, "p": 0.0}

            def boxadd(out, in0, in1, n_elems, allow_pool=True):
                cd = n_elems * _DVE_NS + _DVE_FIX
                cp = n_elems * _POOL_NS + _POOL_FIX
                if not allow_pool or busy["d"] + cd <= busy["p"] + cp:
                    busy["d"] += cd
                    nc.vector.tensor_tensor(out=out, in0=in0, in1=in1, op=ADD)
                else:
                    busy["p"] += cp
                    nc.gpsimd.tensor_tensor(out=out, in0=in0, in1=in1, op=ADD)

            # prefetch all images up front (fresh buffers, no WAR) so the
            # in-order SP queue never delays image 1 behind output DMAs
            x_sbs = []
            for n in range(IMGS_PER_CORE):
                # flat x image + 2 slack elems so full-row matmul spans with
                # a column offset stay in bounds (fp32r matmuls need
                # 2D-collapsible APs: contiguous spans covering whole rows)
                x_sb = xp.tile([CH, 2, XFLAT + 2], F32R, tag="x")
                nc.vector.memset(x_sb[:, 0, XFLAT : XFLAT + 2].bitcast(F32), 0.0)
                nc.vector.memset(x_sb[:, 1, XFLAT : XFLAT + 2].bitcast(F32), 0.0)
                for icx in range(2):
                    nc.sync.dma_start(
                        out=x_sb[:, icx, 0:XFLAT],
                        in_=x_d[n, icx],
                    )
                x_sbs.append(x_sb)
                if n == 0:
                    # remaining constants after unit 0's critical inputs
                    nc.sync.dma_start(
                        out=w_sb[:, :, :, 1, :], in_=w_d[:, :, :, 1, :]
                    )
                    dg_sb = constp.tile([CH, 2 * CH], F16)
                    nc.sync.dma_start(out=dg_sb[:], in_=d_d[:])

            # deferred PE horizontal-FIR bands: (n, ocx, r0, z tile)
            pending = []

            def flush_peh(count):
                for _ in range(min(count, len(pending))):
                    fn_, focx, fr0, fz = pending.pop(0)
                    out_sb = outp.tile([CH, BAND, OUT], F16, tag="out")
                    for g0 in range(0, BAND, GROUP):
                        ps2 = fpsum.tile([CH, GROUP, OUT], F32, tag="fps")
                        for v in range(4):
                            dgi = 0 if v in (0, 3) else 1
                            nc.tensor.matmul(
                                ps2[:],
                                lhsT=dg_sb[:, dgi * CH : (dgi + 1) * CH],
                                rhs=fz[:, g0 : g0 + GROUP, v + 1 : v + 1 + OUT],
                                start=(v == 0),
                                stop=(v == 3),
                            )
                        nc.scalar.copy(
                            out=out_sb[:, g0 : g0 + GROUP, :],
                            in_=ps2[:],
                        )
                    nc.sync.dma_start(
                        out=y_d[fn_, focx, :, fr0 : fr0 + BAND, :],
                        in_=out_sb[:],
                    )

            for n in range(IMGS_PER_CORE):
                x_sb = x_sbs[n]
                for ocx in range(2):
                    unit = 2 * n + ocx
                    # ---------------- stage 1: conv into y1 (fp16) ----------
                    # y1_sb rows: up-row p at index p+1 (rows 0,130,131 zero)
                    # cols: up-col q at index q (col 129 pad, never read)
                    y1_sb = y1p.tile([CH, UP + 3, UP + 1], F16, tag="y1")
                    nc.vector.memset(y1_sb[:, 0:1, 0:UP], 0.0)
                    nc.vector.memset(y1_sb[:, UP + 1 : UP + 3, 0:UP], 0.0)

                    # band-major over up-row chunks: all 4 parities per
                    # m-chunk so the FIR bands can start while later rows
                    # are still being computed
                    for ci, m0 in enumerate(range(0, 65, 7)):
                        if ci >= _FLUSH_START:
                            flush_peh(1)
                        for rho in range(2):
                            for sig in range(2):
                                nm, nn = 65 - rho, 65 - sig
                                if m0 >= nm:
                                    continue
                                dis = (0, 1) if rho == 0 else (0,)
                                djs = (0, 1) if sig == 0 else (0,)
                                r = min(7, nm - m0)
                                ps = cpsum.tile([CH, r, XROW], F32, tag="cps")
                                # icx-major accumulation order: the first
                                # matmuls only need x[icx=0], overlapping
                                # with the x[icx=1] DMA on unit 0
                                mms = [
                                    (di, dj, icx2)
                                    for icx2 in range(2)
                                    for di in dis
                                    for dj in djs
                                ]
                                for k, (di, dj, icx2) in enumerate(mms):
                                    t = (2 * di + rho) * 3 + (2 * dj + sig)
                                    st = (m0 + 1 - di) * XROW + (1 - dj)
                                    nc.tensor.matmul(
                                        ps[:, 0:r, 0:XROW].opt({0}),
                                        lhsT=w_sb[:, icx2, t, ocx, :],
                                        rhs=x_sb[:, icx2, st : st + r * XROW],
                                        start=(k == 0),
                                        stop=(k == len(mms) - 1),
                                    )
                                # strided parity write into the up-grid
                                # (cols nn..65 of each psum row are garbage
                                # from the full-row span and are skipped)
                                nc.scalar.copy(
                                    out=y1_sb[
                                        :,
                                        1 + rho + 2 * m0 : 1 + rho + 2 * (m0 + r) : 2,
                                        sig : sig + 2 * nn : 2,
                                    ],
                                    in_=ps[:, 0:r, 0:nn],
                                )

                    # ---------------- stage 2: FIR box passes per band ------
                    for bi, r0 in enumerate(range(0, OUT, BAND)):
                        pe_h = (unit, bi) in PEH_BANDS
                        pool_ok = (unit, bi) not in _POOL_DENY
                        # vertical: z[r] = y1[r-1] + 3 y1[r] + 3 y1[r+1] + y1[r+2]
                        # (up-row p at y1 index p+1)
                        b1 = firp.tile([CH, BAND + 2, UP + 1], F16, tag="A")
                        boxadd(
                            b1[:, :, 0:UP],
                            y1_sb[:, r0 : r0 + BAND + 2, 0:UP],
                            y1_sb[:, r0 + 1 : r0 + BAND + 3, 0:UP],
                            (BAND + 2) * UP,
                            allow_pool=pool_ok,
                        )
                        b2 = firp.tile([CH, BAND + 1, UP + 1], F16, tag="B")
                        boxadd(
                            b2[:, :, 0:UP],
                            b1[:, 0 : BAND + 1, 0:UP],
                            b1[:, 1 : BAND + 2, 0:UP],
                            (BAND + 1) * UP,
                            allow_pool=pool_ok,
                        )
                        # z cols: up-col q at index q+2 (idx 1 and 131 zero)
                        zpool = zdp if pe_h else zp
                        z = zpool.tile([CH, BAND, UP + 3], F16, tag="z")
                        nc.vector.memset(z[:, :, 1:2], 0.0)
                        nc.vector.memset(z[:, :, UP + 2 : UP + 3], 0.0)
                        boxadd(
                            z[:, :, 2 : UP + 2],
                            b2[:, 0:BAND, 0:UP],
                            b2[:, 1 : BAND + 1, 0:UP],
                            BAND * UP,
                            allow_pool=pool_ok,
                        )

                        if pe_h:
                            # horizontal FIR on TensorE, deferred into the
                            # next unit's conv chunks (or the schedule tail)
                            pending.append((n, ocx, r0, z))
                        else:
                            # horizontal FIR: three 1-col-shift box passes
                            h1 = firp.tile([CH, BAND, UP + 1], F16, tag="A")
                            boxadd(
                                h1[:, :, 0 : UP + 1],
                                z[:, :, 1 : UP + 2],
                                z[:, :, 2 : UP + 3],
                                BAND * (UP + 1),
                                allow_pool=pool_ok,
                            )
                            h2 = firp.tile([CH, BAND, UP], F16, tag="B")
                            boxadd(
                                h2[:, :, 0:UP],
                                h1[:, :, 0:UP],
                                h1[:, :, 1 : UP + 1],
                                BAND * UP,
                                allow_pool=pool_ok,
                            )
                            out_sb = outp.tile([CH, BAND, OUT], F16, tag="out")
                            boxadd(
                                out_sb[:],
                                h2[:, :, 0:OUT],
                                h2[:, :, 1 : OUT + 1],
                                BAND * OUT,
                                allow_pool=pool_ok,
                            )
                            nc.sync.dma_start(
                                out=y_d[n, ocx, :, r0 : r0 + BAND, :],
                                in_=out_sb[:],
                            )
            # schedule tail: the last unit's PE horizontal-FIR bands
            flush_peh(len(pending))
    nc.compile()
    return nc


_NC_CACHE = None


def _get_nc():
    global _NC_CACHE
    if _NC_CACHE is None:
        _NC_CACHE = _build_nc()
    return _NC_CACHE


def kernel(x, w, b):
    x = np.asarray(x, dtype=np.float32)
    w = np.asarray(w, dtype=np.float32)
    b = np.asarray(b, dtype=np.float32)
    # channel-major + zero pad: [N, 2, CH, (H+2)*(W+2)]
    xt = np.zeros((x.shape[0], 2, CH, H + 2, W + 2), dtype=np.float32)
    xt[:, :, :, 1 : H + 1, 1 : W + 1] = x.transpose(0, 3, 1, 2).reshape(
        x.shape[0], 2, CH, H, W
    )
    xt = xt.reshape(x.shape[0], 2, CH, XFLAT)

    # Effective transpose-conv filter, with the separable FIR normalisation
    # (1/4 per axis) folded in.
    Wf = w[::-1, ::-1] / 16.0  # [a, b, ic, oc]
    Wr = Wf.reshape(3, 3, 2, CH, 2, CH)  # a, b, icx, ic, ocx, oc
    w_arr = np.ascontiguousarray(
        Wr.transpose(3, 2, 0, 1, 4, 5).reshape(CH, 2, 9, 2, CH)
    )
    eye = np.eye(CH, dtype=np.float16)
    dg = np.ascontiguousarray(np.concatenate([eye, 3.0 * eye], axis=1))

    in_maps = [
        {
            "x": np.ascontiguousarray(xt[IMGS_PER_CORE * c : IMGS_PER_CORE * (c + 1)]),
            "wt": w_arr,
            "dg": dg,
        }
        for c in range(N_CORES)
    ]
    nc = _get_nc()
    res = run_bass_kernel_spmd(nc, in_maps, core_ids=list(range(N_CORES)))
    # [n, 2, CH, r, s] fp16 -> [n, r, s, 2*CH] fp32 + bias
    y = np.concatenate([res.results[c]["y"] for c in range(N_CORES)], axis=0)
    y = y.reshape(-1, C, OUT, OUT).transpose(0, 2, 3, 1).astype(np.float32)
    y += b.reshape(1, 1, 1, C)
    return np.ascontiguousarray(y)


if __name__ == "__main__":
    rng = np.random.default_rng(0)
    x = rng.standard_normal((16, 64, 64, 256), dtype=np.float32)
    w = rng.standard_normal((3, 3, 256, 256), dtype=np.float32) * 0.02
    b = np.zeros((256,), dtype=np.float32)
    y = kernel(x, w, b)
    print("out:", y.shape, y.dtype)


# revision 14
# speedup vs baseline: 1.2866x; 1.0807x over previous
"""Trainium2 Bass kernel for StyleGAN2-style fused upsample(x2)+conv3x3+FIR.

Reference computation (per image):
    y1 = conv_transpose(x, w', stride=2, VALID)          # [129,129,256]
    y  = depthwise_FIR_4x4(pad(y1,1)) + b                # [128,128,256]

Implementation strategy (per NeuronCore, data-parallel over batch 16 -> 8
cores x 2 images; each image processed as 2 units of 128 output channels):

  Stage 1 (TensorE): subpixel decomposition of the stride-2 transpose conv.
    Output parity (rho,sig) of the upsampled grid is a stride-1 VALID conv
    of the zero-padded x with taps W[a,b], a = 2*di+rho.  Matmuls contract
    over in-channels (128 per chunk), fp32 data bitcast to float32r (FP22
    multiply, full PE rate at free-dim >= 256).  ScalarE interleaves the
    parity grids into a dense fp16 up-grid y1 (strided psum->SBUF copies).

  Stage 2: separable FIR as six 2-tap box passes ([1,3,3,1] = [1,1]^*3 per
    axis; the 1/16 normalisation is folded into W).  Three vertical passes
    (whole-row shifts) then three horizontal passes (1-col shifts), all
    fp16 tensor_tensor adds distributed between VectorE and GpSimd by a
    static load-balancing heuristic.  For the last unit the horizontal FIR
    runs on TensorE instead (4 accumulating diagonal matmuls per row
    group) -- the conv work is finished by then, so this shortens the tail.

  Output is written fp16 (channel-major [n, ocx, ch, r, s]); the host
  upcasts, transposes back to NHWC and adds the bias.
"""

import sys

sys.path.insert(0, "/opt/trn_rl_repo")

import numpy as np

import concourse.bass as bass  # noqa: F401  (registers engine classes)
import concourse.mybir as mybir
import concourse.tile as tile
from concourse import bacc
from concourse.bass_utils import run_bass_kernel_spmd

F32 = mybir.dt.float32
F32R = mybir.dt.float32r
F16 = mybir.dt.float16
ADD = mybir.AluOpType.add

N_CORES = 8
IMGS_PER_CORE = 2
H = W = 64          # input spatial
UP = 129            # upsampled grid (conv_transpose output)
OUT = 128           # final spatial
C = 256             # channels
CH = 128            # channels per partition chunk
BAND = 16           # FIR band rows (8 bands per unit)
GROUP = 4           # col-FIR psum group rows (4*128 = 512 free)

XROW = W + 2          # 66: padded x row length
XFLAT = (H + 2) * XROW  # 4356: flat padded image

# (unit, band) pairs whose horizontal FIR runs on TensorE.  Emission of the
# matmuls is deferred into the next unit's conv chunks (the z tile is ready
# by then), so the in-order PE queue never stalls; the last unit's PEH bands
# are emitted at the very end, where PE is otherwise idle.
PEH_BANDS = {
    (0, 5), (0, 6), (0, 7),
    (1, 5), (1, 6), (1, 7),
    (2, 5), (2, 6), (2, 7),
    (3, 3), (3, 4), (3, 5), (3, 6), (3, 7),
}
# first conv m-chunk (of 10) at which deferred PEH bands may be flushed
_FLUSH_START = 6


def _pat(unit, band):
    """Engine per FIR pass (V1,V2,V3,H1,H2,H3): 'd'=VectorE, 'p'=GpSimd.
    Once a band's pass runs on GpSimd all later passes of that band stay
    there, so VectorE's in-order queue never waits on GpSimd."""
    if (unit, band) in PEH_BANDS:
        return "ddd"
    if band == 1:
        return "dppppp"
    if band == 4 and unit < 3:
        return "dddppp"
    return "dddddd"



def _build_nc():
    nc = bacc.Bacc("TRN2", target_bir_lowering=False)

    # x arrives host-padded to 66x66 (zero border) and channel-major
    # [n, icx, ch, h*w] so each partition's DMA run is contiguous
    x_d = nc.dram_tensor("x", [IMGS_PER_CORE, 2, CH, XFLAT], F32R, kind="ExternalInput")
    # Pre-arranged conv taps: [ic_part, icx, tap(a*3+b), ocx, oc]
    w_d = nc.dram_tensor("wt", [CH, 2, 9, 2, CH], F32R, kind="ExternalInput")
    # Diagonal FIR weights, fp16: [:,0:128] = I, [:,128:256] = 3I
    d_d = nc.dram_tensor("dg", [CH, 2 * CH], F16, kind="ExternalInput")
    # channel-major fp16 output [n, ocx, ch, r, s]; host transposes to NHWC
    y_d = nc.dram_tensor("y", [IMGS_PER_CORE, 2, CH, OUT, OUT], F16, kind="ExternalOutput")

    with tile.TileContext(nc) as tc:
        with (
            tc.tile_pool(name="const", bufs=1) as constp,
            tc.tile_pool(name="xp", bufs=2) as xp,
            tc.tile_pool(name="y1p", bufs=1) as y1p,
            tc.tile_pool(name="firp", bufs=3) as firp,
            tc.tile_pool(name="zp", bufs=4) as zp,
            tc.tile_pool(name="zdp", bufs=5) as zdp,
            tc.tile_pool(name="outp", bufs=3) as outp,
            tc.tile_pool(name="cpsum", bufs=4, space="PSUM") as cpsum,
            tc.tile_pool(name="fpsum", bufs=4, space="PSUM") as fpsum,
        ):
            w_sb = constp.tile([CH, 2, 9, 2, CH], F32R)
            # ocx=0 taps first so unit 0's matmuls can start sooner
            nc.sync.dma_start(out=w_sb[:, :, :, 0, :], in_=w_d[:, :, :, 0, :])

            def boxadd(eng, out, in0, in1):
                e = nc.vector if eng == "d" else nc.gpsimd
                e.tensor_tensor(out=out, in0=in0, in1=in1, op=ADD)

            # prefetch all images up front (fresh buffers, no WAR) so the
            # in-order SP queue never delays image 1 behind output DMAs
            x_sbs = []
            for n in range(IMGS_PER_CORE):
                # flat x image + 2 slack elems so full-row matmul spans with
                # a column offset stay in bounds (fp32r matmuls need
                # 2D-collapsible APs: contiguous spans covering whole rows)
                x_sb = xp.tile([CH, 2, XFLAT + 2], F32R, tag="x")
                nc.vector.memset(x_sb[:, 0, XFLAT : XFLAT + 2].bitcast(F32), 0.0)
                nc.vector.memset(x_sb[:, 1, XFLAT : XFLAT + 2].bitcast(F32), 0.0)
                for icx in range(2):
                    nc.sync.dma_start(
                        out=x_sb[:, icx, 0:XFLAT],
                        in_=x_d[n, icx],
                    )
                x_sbs.append(x_sb)
                if n == 0:
                    # remaining constants after unit 0's critical inputs
                    nc.sync.dma_start(
                        out=w_sb[:, :, :, 1, :], in_=w_d[:, :, :, 1, :]
                    )
                    dg_sb = constp.tile([CH, 2 * CH], F16)
                    nc.sync.dma_start(out=dg_sb[:], in_=d_d[:])

            # single y1 buffer reused by all units; the zero halo rows are
            # never overwritten, so set them once
            y1_sb = y1p.tile([CH, UP + 3, UP + 1], F16, tag="y1")
            nc.vector.memset(y1_sb[:, 0:1, 0:UP], 0.0)
            nc.vector.memset(y1_sb[:, UP + 1 : UP + 3, 0:UP], 0.0)

            # deferred PE horizontal-FIR bands: (n, ocx, r0, z tile)
            pending = []

            def flush_peh(count):
                for _ in range(min(count, len(pending))):
                    fn_, focx, fr0, fz = pending.pop(0)
                    out_sb = outp.tile([CH, BAND, OUT], F16, tag="out")
                    for g0 in range(0, BAND, GROUP):
                        ps2 = fpsum.tile([CH, GROUP, OUT], F32, tag="fps")
                        for v in range(4):
                            dgi = 0 if v in (0, 3) else 1
                            nc.tensor.matmul(
                                ps2[:],
                                lhsT=dg_sb[:, dgi * CH : (dgi + 1) * CH],
                                rhs=fz[:, g0 : g0 + GROUP, v + 1 : v + 1 + OUT],
                                start=(v == 0),
                                stop=(v == 3),
                            )
                        nc.scalar.copy(
                            out=out_sb[:, g0 : g0 + GROUP, :],
                            in_=ps2[:],
                        )
                    nc.sync.dma_start(
                        out=y_d[fn_, focx, :, fr0 : fr0 + BAND, :],
                        in_=out_sb[:],
                    )

            for n in range(IMGS_PER_CORE):
                x_sb = x_sbs[n]
                for ocx in range(2):
                    unit = 2 * n + ocx
                    # ---------------- stage 1: conv into y1 (fp16) ----------
                    # y1_sb rows: up-row p at index p+1 (rows 0,130,131 zero)
                    # cols: up-col q at index q (col 129 pad, never read)
                    # band-major over up-row chunks: all 4 parities per
                    # m-chunk so the FIR bands can start while later rows
                    # are still being computed
                    for ci, m0 in enumerate(range(0, 65, 7)):
                        if ci >= _FLUSH_START:
                            flush_peh(1)
                        for rho in range(2):
                            for sig in range(2):
                                nm, nn = 65 - rho, 65 - sig
                                if m0 >= nm:
                                    continue
                                dis = (0, 1) if rho == 0 else (0,)
                                djs = (0, 1) if sig == 0 else (0,)
                                r = min(7, nm - m0)
                                ps = cpsum.tile([CH, r, XROW], F32, tag="cps")
                                # icx-major accumulation order: the first
                                # matmuls only need x[icx=0], overlapping
                                # with the x[icx=1] DMA on unit 0
                                mms = [
                                    (di, dj, icx2)
                                    for icx2 in range(2)
                                    for di in dis
                                    for dj in djs
                                ]
                                for k, (di, dj, icx2) in enumerate(mms):
                                    t = (2 * di + rho) * 3 + (2 * dj + sig)
                                    st = (m0 + 1 - di) * XROW + (1 - dj)
                                    nc.tensor.matmul(
                                        ps[:, 0:r, 0:XROW].opt({0}),
                                        lhsT=w_sb[:, icx2, t, ocx, :],
                                        rhs=x_sb[:, icx2, st : st + r * XROW],
                                        start=(k == 0),
                                        stop=(k == len(mms) - 1),
                                    )
                                # strided parity write into the up-grid
                                # (cols nn..65 of each psum row are garbage
                                # from the full-row span and are skipped)
                                nc.scalar.copy(
                                    out=y1_sb[
                                        :,
                                        1 + rho + 2 * m0 : 1 + rho + 2 * (m0 + r) : 2,
                                        sig : sig + 2 * nn : 2,
                                    ],
                                    in_=ps[:, 0:r, 0:nn],
                                )

                    # ---------------- stage 2: FIR box passes per band ------
                    for bi, r0 in enumerate(range(0, OUT, BAND)):
                        pe_h = (unit, bi) in PEH_BANDS
                        pat = _pat(unit, bi)
                        # vertical: z[r] = y1[r-1] + 3 y1[r] + 3 y1[r+1] + y1[r+2]
                        # (up-row p at y1 index p+1)
                        b1 = firp.tile([CH, BAND + 2, UP + 1], F16, tag="A")
                        boxadd(
                            pat[0],
                            b1[:, :, 0:UP],
                            y1_sb[:, r0 : r0 + BAND + 2, 0:UP],
                            y1_sb[:, r0 + 1 : r0 + BAND + 3, 0:UP],
                        )
                        b2 = firp.tile([CH, BAND + 1, UP + 1], F16, tag="B")
                        boxadd(
                            pat[1],
                            b2[:, :, 0:UP],
                            b1[:, 0 : BAND + 1, 0:UP],
                            b1[:, 1 : BAND + 2, 0:UP],
                        )
                        # z cols: up-col q at index q+2 (idx 1 and 131 zero)
                        zpool = zdp if pe_h else zp
                        z = zpool.tile([CH, BAND, UP + 3], F16, tag="z")
                        nc.vector.memset(z[:, :, 1:2], 0.0)
                        nc.vector.memset(z[:, :, UP + 2 : UP + 3], 0.0)
                        boxadd(
                            pat[2],
                            z[:, :, 2 : UP + 2],
                            b2[:, 0:BAND, 0:UP],
                            b2[:, 1 : BAND + 1, 0:UP],
                        )

                        if pe_h:
                            # horizontal FIR on TensorE, deferred into the
                            # next unit's conv chunks (or the schedule tail)
                            pending.append((n, ocx, r0, z))
                        else:
                            # horizontal FIR: three 1-col-shift box passes
                            h1 = firp.tile([CH, BAND, UP + 1], F16, tag="A")
                            boxadd(
                                pat[3],
                                h1[:, :, 0 : UP + 1],
                                z[:, :, 1 : UP + 2],
                                z[:, :, 2 : UP + 3],
                            )
                            h2 = firp.tile([CH, BAND, UP], F16, tag="B")
                            boxadd(
                                pat[4],
                                h2[:, :, 0:UP],
                                h1[:, :, 0:UP],
                                h1[:, :, 1 : UP + 1],
                            )
                            out_sb = outp.tile([CH, BAND, OUT], F16, tag="out")
                            boxadd(
                                pat[5],
                                out_sb[:],
                                h2[:, :, 0:OUT],
                                h2[:, :, 1 : OUT + 1],
                            )
                            nc.sync.dma_start(
                                out=y_d[n, ocx, :, r0 : r0 + BAND, :],
                                in_=out_sb[:],
                            )
            # schedule tail: the last unit's PE horizontal-FIR bands
            flush_peh(len(pending))
    nc.compile()
    return nc


_NC_CACHE = None


def _get_nc():
    global _NC_CACHE
    if _NC_CACHE is None:
        _NC_CACHE = _build_nc()
    return _NC_CACHE


def kernel(x, w, b):
    x = np.asarray(x, dtype=np.float32)
    w = np.asarray(w, dtype=np.float32)
    b = np.asarray(b, dtype=np.float32)
    # channel-major + zero pad: [N, 2, CH, (H+2)*(W+2)]
    xt = np.zeros((x.shape[0], 2, CH, H + 2, W + 2), dtype=np.float32)
    xt[:, :, :, 1 : H + 1, 1 : W + 1] = x.transpose(0, 3, 1, 2).reshape(
        x.shape[0], 2, CH, H, W
    )
    xt = xt.reshape(x.shape[0], 2, CH, XFLAT)

    # Effective transpose-conv filter, with the separable FIR normalisation
    # (1/4 per axis) folded in.
    Wf = w[::-1, ::-1] / 16.0  # [a, b, ic, oc]
    Wr = Wf.reshape(3, 3, 2, CH, 2, CH)  # a, b, icx, ic, ocx, oc
    w_arr = np.ascontiguousarray(
        Wr.transpose(3, 2, 0, 1, 4, 5).reshape(CH, 2, 9, 2, CH)
    )
    eye = np.eye(CH, dtype=np.float16)
    dg = np.ascontiguousarray(np.concatenate([eye, 3.0 * eye], axis=1))

    in_maps = [
        {
            "x": np.ascontiguousarray(xt[IMGS_PER_CORE * c : IMGS_PER_CORE * (c + 1)]),
            "wt": w_arr,
            "dg": dg,
        }
        for c in range(N_CORES)
    ]
    nc = _get_nc()
    res = run_bass_kernel_spmd(nc, in_maps, core_ids=list(range(N_CORES)))
    # [n, 2, CH, r, s] fp16 -> [n, r, s, 2*CH] fp32 + bias
    y = np.concatenate([res.results[c]["y"] for c in range(N_CORES)], axis=0)
    y = y.reshape(-1, C, OUT, OUT).transpose(0, 2, 3, 1).astype(np.float32)
    y += b.reshape(1, 1, 1, C)
    return np.ascontiguousarray(y)


if __name__ == "__main__":
    rng = np.random.default_rng(0)
    x = rng.standard_normal((16, 64, 64, 256), dtype=np.float32)
    w = rng.standard_normal((3, 3, 256, 256), dtype=np.float32) * 0.02
    b = np.zeros((256,), dtype=np.float32)
    y = kernel(x, w, b)
    print("out:", y.shape, y.dtype)


# revision 16
# speedup vs baseline: 1.3397x; 1.0412x over previous
"""Trainium2 Bass kernel for StyleGAN2-style fused upsample(x2)+conv3x3+FIR.

Reference computation (per image):
    y1 = conv_transpose(x, w', stride=2, VALID)          # [129,129,256]
    y  = depthwise_FIR_4x4(pad(y1,1)) + b                # [128,128,256]

Implementation strategy (per NeuronCore, data-parallel over batch 16 -> 8
cores x 2 images; each image processed as 2 units of 128 output channels):

  Stage 1 (TensorE): subpixel decomposition of the stride-2 transpose conv.
    Output parity (rho,sig) of the upsampled grid is a stride-1 VALID conv
    of the zero-padded x with taps W[a,b], a = 2*di+rho.  Matmuls contract
    over in-channels (128 per chunk), fp32 data bitcast to float32r (FP22
    multiply, full PE rate at free-dim >= 256).  ScalarE interleaves the
    parity grids into a dense fp16 up-grid y1 (strided psum->SBUF copies).

  Stage 2: separable FIR as six 2-tap box passes ([1,3,3,1] = [1,1]^*3 per
    axis; the 1/16 normalisation is folded into W).  Three vertical passes
    (whole-row shifts) then three horizontal passes (1-col shifts), all
    fp16 tensor_tensor adds distributed between VectorE and GpSimd by a
    static load-balancing heuristic.  For the last unit the horizontal FIR
    runs on TensorE instead (4 accumulating diagonal matmuls per row
    group) -- the conv work is finished by then, so this shortens the tail.

  Output is written fp16 (channel-major [n, ocx, ch, r, s]); the host
  upcasts, transposes back to NHWC and adds the bias.
"""

import sys

sys.path.insert(0, "/opt/trn_rl_repo")

import numpy as np

import concourse.bass as bass  # noqa: F401  (registers engine classes)
import concourse.mybir as mybir
import concourse.tile as tile
from concourse import bacc
from concourse.bass_utils import run_bass_kernel_spmd

F32 = mybir.dt.float32
F32R = mybir.dt.float32r
F16 = mybir.dt.float16
ADD = mybir.AluOpType.add

N_CORES = 8
IMGS_PER_CORE = 2
H = W = 64          # input spatial
UP = 129            # upsampled grid (conv_transpose output)
OUT = 128           # final spatial
C = 256             # channels
CH = 128            # channels per partition chunk
BAND = 16           # FIR band rows (8 bands per unit)
GROUP = 4           # col-FIR psum group rows (4*128 = 512 free)

XROW = W + 2          # 66: padded x row length
XFLAT = (H + 2) * XROW  # 4356: flat padded image

# (unit, band) pairs whose horizontal FIR runs on TensorE.  Emission of the
# matmuls is deferred into the next unit's conv chunks (the z tile is ready
# by then), so the in-order PE queue never stalls; the last unit's PEH bands
# are emitted at the very end, where PE is otherwise idle.
PEH_BANDS = {
    (0, 5), (0, 6), (0, 7),
    (1, 5), (1, 6), (1, 7),
    (2, 5), (2, 6), (2, 7),
    (3, 3), (3, 4), (3, 5), (3, 6), (3, 7),
}
# first conv m-chunk (of 10) at which deferred PEH bands may be flushed
_FLUSH_START = 6


def _pat(unit, band):
    """Engine per FIR pass (V1,V2,V3,H1,H2,H3): 'd'=VectorE, 'p'=GpSimd.
    Once a band's pass runs on GpSimd all later passes of that band stay
    there, so VectorE's in-order queue never waits on GpSimd."""
    if (unit, band) in PEH_BANDS:
        return "ddd"
    if band == 1:
        return "dppppp"
    if band == 3 and unit < 3:
        return "dddppp"
    return "dddddd"


# bands whose horizontal passes are emitted at the START of the next unit's
# section: they have no conv dependency left, so they fill the boundary
# bubble where VectorE/GpSimd would otherwise wait for the next unit's
# first psum copies
_DEFER_H = {3, 4}



def _build_nc():
    nc = bacc.Bacc("TRN2", target_bir_lowering=False)

    # x arrives host-padded to 66x66 (zero border) and channel-major
    # [n, icx, ch, h*w] so each partition's DMA run is contiguous
    x_d = nc.dram_tensor("x", [IMGS_PER_CORE, 2, CH, XFLAT], F32R, kind="ExternalInput")
    # Pre-arranged conv taps: [ic_part, icx, tap(a*3+b), ocx, oc]
    w_d = nc.dram_tensor("wt", [CH, 2, 9, 2, CH], F32R, kind="ExternalInput")
    # Diagonal FIR weights, fp16: [:,0:128] = I, [:,128:256] = 3I
    d_d = nc.dram_tensor("dg", [CH, 2 * CH], F16, kind="ExternalInput")
    # channel-major fp16 output [n, ocx, ch, r, s]; host transposes to NHWC
    y_d = nc.dram_tensor("y", [IMGS_PER_CORE, 2, CH, OUT, OUT], F16, kind="ExternalOutput")

    with tile.TileContext(nc) as tc:
        with (
            tc.tile_pool(name="const", bufs=1) as constp,
            tc.tile_pool(name="xp", bufs=2) as xp,
            tc.tile_pool(name="y1p", bufs=1) as y1p,
            tc.tile_pool(name="firp", bufs=3) as firp,
            tc.tile_pool(name="zp", bufs=4) as zp,
            tc.tile_pool(name="zdp", bufs=5) as zdp,
            tc.tile_pool(name="outp", bufs=3) as outp,
            tc.tile_pool(name="cpsum", bufs=5, space="PSUM") as cpsum,
            tc.tile_pool(name="fpsum", bufs=3, space="PSUM") as fpsum,
        ):
            w_sb = constp.tile([CH, 2, 9, 2, CH], F32R)
            # ocx=0 taps first so unit 0's matmuls can start sooner
            nc.sync.dma_start(out=w_sb[:, :, :, 0, :], in_=w_d[:, :, :, 0, :])

            def boxadd(eng, out, in0, in1):
                e = nc.vector if eng == "d" else nc.gpsimd
                e.tensor_tensor(out=out, in0=in0, in1=in1, op=ADD)

            # prefetch all images up front (fresh buffers, no WAR) so the
            # in-order SP queue never delays image 1 behind output DMAs
            x_sbs = []
            for n in range(IMGS_PER_CORE):
                # flat x image + 2 slack elems so full-row matmul spans with
                # a column offset stay in bounds (fp32r matmuls need
                # 2D-collapsible APs: contiguous spans covering whole rows)
                x_sb = xp.tile([CH, 2, XFLAT + 2], F32R, tag="x")
                nc.vector.memset(x_sb[:, 0, XFLAT : XFLAT + 2].bitcast(F32), 0.0)
                nc.vector.memset(x_sb[:, 1, XFLAT : XFLAT + 2].bitcast(F32), 0.0)
                for icx in range(2):
                    nc.sync.dma_start(
                        out=x_sb[:, icx, 0:XFLAT],
                        in_=x_d[n, icx],
                    )
                x_sbs.append(x_sb)
                if n == 0:
                    # remaining constants after unit 0's critical inputs
                    nc.sync.dma_start(
                        out=w_sb[:, :, :, 1, :], in_=w_d[:, :, :, 1, :]
                    )
                    dg_sb = constp.tile([CH, 2 * CH], F16)
                    nc.sync.dma_start(out=dg_sb[:], in_=d_d[:])

            # single y1 buffer reused by all units; the zero halo rows are
            # never overwritten, so set them once
            y1_sb = y1p.tile([CH, UP + 3, UP + 1], F16, tag="y1")
            nc.vector.memset(y1_sb[:, 0:1, 0:UP], 0.0)
            nc.vector.memset(y1_sb[:, UP + 1 : UP + 3, 0:UP], 0.0)

            # deferred PE horizontal-FIR bands: (n, ocx, r0, z tile)
            pending = []

            def flush_peh(count):
                for _ in range(min(count, len(pending))):
                    fn_, focx, fr0, fz = pending.pop(0)
                    out_sb = outp.tile([CH, BAND, OUT], F16, tag="out")
                    for g0 in range(0, BAND, GROUP):
                        ps2 = fpsum.tile([CH, GROUP, OUT], F32, tag="fps")
                        for v in range(4):
                            dgi = 0 if v in (0, 3) else 1
                            nc.tensor.matmul(
                                ps2[:],
                                lhsT=dg_sb[:, dgi * CH : (dgi + 1) * CH],
                                rhs=fz[:, g0 : g0 + GROUP, v + 1 : v + 1 + OUT],
                                start=(v == 0),
                                stop=(v == 3),
                            )
                        nc.scalar.copy(
                            out=out_sb[:, g0 : g0 + GROUP, :],
                            in_=ps2[:],
                        )
                    nc.sync.dma_start(
                        out=y_d[fn_, focx, :, fr0 : fr0 + BAND, :],
                        in_=out_sb[:],
                    )

            # horizontal FIR box passes for one band + output DMA
            def emit_h(pat, z, hn, hocx, hr0):
                h1 = firp.tile([CH, BAND, UP + 1], F16, tag="A")
                boxadd(
                    pat[3],
                    h1[:, :, 0 : UP + 1],
                    z[:, :, 1 : UP + 2],
                    z[:, :, 2 : UP + 3],
                )
                h2 = firp.tile([CH, BAND, UP], F16, tag="B")
                boxadd(
                    pat[4],
                    h2[:, :, 0:UP],
                    h1[:, :, 0:UP],
                    h1[:, :, 1 : UP + 1],
                )
                out_sb = outp.tile([CH, BAND, OUT], F16, tag="out")
                boxadd(
                    pat[5],
                    out_sb[:],
                    h2[:, :, 0:OUT],
                    h2[:, :, 1 : OUT + 1],
                )
                nc.sync.dma_start(
                    out=y_d[hn, hocx, :, hr0 : hr0 + BAND, :],
                    in_=out_sb[:],
                )

            deferred_h = []

            for n in range(IMGS_PER_CORE):
                x_sb = x_sbs[n]
                for ocx in range(2):
                    unit = 2 * n + ocx
                    # boundary-bubble filler: the previous unit's deferred
                    # horizontal bands (no conv dependency left)
                    for args in deferred_h:
                        emit_h(*args)
                    deferred_h.clear()
                    # ---------------- stage 1: conv into y1 (fp16) ----------
                    # y1_sb rows: up-row p at index p+1 (rows 0,130,131 zero)
                    # cols: up-col q at index q (col 129 pad, never read)
                    # band-major over up-row chunks: all 4 parities per
                    # m-chunk so the FIR bands can start while later rows
                    # are still being computed
                    for ci, m0 in enumerate(range(0, 65, 7)):
                        if ci >= _FLUSH_START:
                            flush_peh(1)
                        for rho in range(2):
                            for sig in range(2):
                                nm, nn = 65 - rho, 65 - sig
                                if m0 >= nm:
                                    continue
                                dis = (0, 1) if rho == 0 else (0,)
                                djs = (0, 1) if sig == 0 else (0,)
                                r = min(7, nm - m0)
                                ps = cpsum.tile([CH, r, XROW], F32, tag="cps")
                                # icx-major accumulation order: the first
                                # matmuls only need x[icx=0], overlapping
                                # with the x[icx=1] DMA on unit 0
                                mms = [
                                    (di, dj, icx2)
                                    for icx2 in range(2)
                                    for di in dis
                                    for dj in djs
                                ]
                                for k, (di, dj, icx2) in enumerate(mms):
                                    t = (2 * di + rho) * 3 + (2 * dj + sig)
                                    st = (m0 + 1 - di) * XROW + (1 - dj)
                                    nc.tensor.matmul(
                                        ps[:, 0:r, 0:XROW].opt({0}),
                                        lhsT=w_sb[:, icx2, t, ocx, :],
                                        rhs=x_sb[:, icx2, st : st + r * XROW],
                                        start=(k == 0),
                                        stop=(k == len(mms) - 1),
                                    )
                                # strided parity write into the up-grid
                                # (cols nn..65 of each psum row are garbage
                                # from the full-row span and are skipped)
                                nc.scalar.copy(
                                    out=y1_sb[
                                        :,
                                        1 + rho + 2 * m0 : 1 + rho + 2 * (m0 + r) : 2,
                                        sig : sig + 2 * nn : 2,
                                    ],
                                    in_=ps[:, 0:r, 0:nn],
                                )

                    # ---------------- stage 2: FIR box passes per band ------
                    for bi, r0 in enumerate(range(0, OUT, BAND)):
                        pe_h = (unit, bi) in PEH_BANDS
                        pat = _pat(unit, bi)
                        # vertical: z[r] = y1[r-1] + 3 y1[r] + 3 y1[r+1] + y1[r+2]
                        # (up-row p at y1 index p+1)
                        b1 = firp.tile([CH, BAND + 2, UP + 1], F16, tag="A")
                        boxadd(
                            pat[0],
                            b1[:, :, 0:UP],
                            y1_sb[:, r0 : r0 + BAND + 2, 0:UP],
                            y1_sb[:, r0 + 1 : r0 + BAND + 3, 0:UP],
                        )
                        b2 = firp.tile([CH, BAND + 1, UP + 1], F16, tag="B")
                        boxadd(
                            pat[1],
                            b2[:, :, 0:UP],
                            b1[:, 0 : BAND + 1, 0:UP],
                            b1[:, 1 : BAND + 2, 0:UP],
                        )
                        # z cols: up-col q at index q+2 (idx 1 and 131 zero)
                        zpool = zdp if pe_h else zp
                        z = zpool.tile([CH, BAND, UP + 3], F16, tag="z")
                        nc.vector.memset(z[:, :, 1:2], 0.0)
                        nc.vector.memset(z[:, :, UP + 2 : UP + 3], 0.0)
                        boxadd(
                            pat[2],
                            z[:, :, 2 : UP + 2],
                            b2[:, 0:BAND, 0:UP],
                            b2[:, 1 : BAND + 1, 0:UP],
                        )

                        if pe_h:
                            # horizontal FIR on TensorE, deferred into the
                            # next unit's conv chunks (or the schedule tail)
                            pending.append((n, ocx, r0, z))
                        elif bi in _DEFER_H and unit < 3:
                            deferred_h.append((pat, z, n, ocx, r0))
                        else:
                            emit_h(pat, z, n, ocx, r0)
            # schedule tail: any deferred bands, then the last unit's PE
            # horizontal-FIR bands
            for args in deferred_h:
                emit_h(*args)
            deferred_h.clear()
            flush_peh(len(pending))
    nc.compile()
    return nc


_NC_CACHE = None


def _get_nc():
    global _NC_CACHE
    if _NC_CACHE is None:
        _NC_CACHE = _build_nc()
    return _NC_CACHE


def kernel(x, w, b):
    x = np.asarray(x, dtype=np.float32)
    w = np.asarray(w, dtype=np.float32)
    b = np.asarray(b, dtype=np.float32)
    # channel-major + zero pad: [N, 2, CH, (H+2)*(W+2)]
    xt = np.zeros((x.shape[0], 2, CH, H + 2, W + 2), dtype=np.float32)
    xt[:, :, :, 1 : H + 1, 1 : W + 1] = x.transpose(0, 3, 1, 2).reshape(
        x.shape[0], 2, CH, H, W
    )
    xt = xt.reshape(x.shape[0], 2, CH, XFLAT)

    # Effective transpose-conv filter, with the separable FIR normalisation
    # (1/4 per axis) folded in.
    Wf = w[::-1, ::-1] / 16.0  # [a, b, ic, oc]
    Wr = Wf.reshape(3, 3, 2, CH, 2, CH)  # a, b, icx, ic, ocx, oc
    w_arr = np.ascontiguousarray(
        Wr.transpose(3, 2, 0, 1, 4, 5).reshape(CH, 2, 9, 2, CH)
    )
    eye = np.eye(CH, dtype=np.float16)
    dg = np.ascontiguousarray(np.concatenate([eye, 3.0 * eye], axis=1))

    in_maps = [
        {
            "x": np.ascontiguousarray(xt[IMGS_PER_CORE * c : IMGS_PER_CORE * (c + 1)]),
            "wt": w_arr,
            "dg": dg,
        }
        for c in range(N_CORES)
    ]
    nc = _get_nc()
    res = run_bass_kernel_spmd(nc, in_maps, core_ids=list(range(N_CORES)))
    # [n, 2, CH, r, s] fp16 -> [n, r, s, 2*CH] fp32 + bias
    y = np.concatenate([res.results[c]["y"] for c in range(N_CORES)], axis=0)
    y = y.reshape(-1, C, OUT, OUT).transpose(0, 2, 3, 1).astype(np.float32)
    y += b.reshape(1, 1, 1, C)
    return np.ascontiguousarray(y)


if __name__ == "__main__":
    rng = np.random.default_rng(0)
    x = rng.standard_normal((16, 64, 64, 256), dtype=np.float32)
    w = rng.standard_normal((3, 3, 256, 256), dtype=np.float32) * 0.02
    b = np.zeros((256,), dtype=np.float32)
    y = kernel(x, w, b)
    print("out:", y.shape, y.dtype)


# revision 18
# speedup vs baseline: 1.4991x; 1.1190x over previous
"""Trainium2 Bass kernel for StyleGAN2-style fused upsample(x2)+conv3x3+FIR.

Reference computation (per image):
    y1 = conv_transpose(x, w', stride=2, VALID)          # [129,129,256]
    y  = depthwise_FIR_4x4(pad(y1,1)) + b                # [128,128,256]

Implementation strategy (per NeuronCore, data-parallel over batch 16 -> 8
cores x 2 images; each image processed as 2 units of 128 output channels):

  Stage 1 (TensorE): subpixel decomposition of the stride-2 transpose conv.
    Output parity (rho,sig) of the upsampled grid is a stride-1 VALID conv
    of the zero-padded x with taps W[a,b], a = 2*di+rho.  Matmuls contract
    over in-channels (128 per chunk), fp32 data bitcast to float32r (FP22
    multiply, full PE rate at free-dim >= 256).  ScalarE interleaves the
    parity grids into a dense fp16 up-grid y1 (strided psum->SBUF copies).

  Stage 2: separable FIR as six 2-tap box passes ([1,3,3,1] = [1,1]^*3 per
    axis; the 1/16 normalisation is folded into W).  Three vertical passes
    (whole-row shifts) then three horizontal passes (1-col shifts), all
    fp16 tensor_tensor adds distributed between VectorE and GpSimd by a
    static load-balancing heuristic.  For the last unit the horizontal FIR
    runs on TensorE instead (4 accumulating diagonal matmuls per row
    group) -- the conv work is finished by then, so this shortens the tail.

  Output is written fp16 (channel-major [n, ocx, ch, r, s]); the host
  upcasts, transposes back to NHWC and adds the bias.
"""

import sys

sys.path.insert(0, "/opt/trn_rl_repo")

import numpy as np

import concourse.bass as bass  # noqa: F401  (registers engine classes)
import concourse.mybir as mybir
import concourse.tile as tile
from concourse import bacc
from concourse.bass_utils import run_bass_kernel_spmd

F32 = mybir.dt.float32
F32R = mybir.dt.float32r
F16 = mybir.dt.float16
ADD = mybir.AluOpType.add

N_CORES = 8
IMGS_PER_CORE = 2
H = W = 64          # input spatial
UP = 129            # upsampled grid (conv_transpose output)
OUT = 128           # final spatial
C = 256             # channels
CH = 128            # channels per partition chunk
BAND = 16           # FIR band rows (8 bands per unit)
GROUP = 4           # col-FIR psum group rows (4*128 = 512 free)

XROW = W + 2          # 66: padded x row length
XFLAT = (H + 2) * XROW  # 4356: flat padded image

# (unit, band) pairs whose horizontal FIR runs on TensorE.  Emission of the
# matmuls is deferred into the next unit's conv chunks (the z tile is ready
# by then), so the in-order PE queue never stalls; the last unit's PEH bands
# are emitted at the very end, where PE is otherwise idle.
PEH_BANDS = {
    (0, 5), (0, 6), (0, 7),
    (1, 5), (1, 6), (1, 7),
    (2, 5), (2, 6), (2, 7),
    (3, 3), (3, 4), (3, 5), (3, 6), (3, 7),
}
# first conv m-chunk (of 10) at which deferred PEH bands may be flushed
_FLUSH_START = 6


def _pat(unit, band):
    """Engine per FIR pass (V1,V2,V3,H1,H2,H3): 'd'=VectorE, 'p'=GpSimd.
    Once a band's pass runs on GpSimd all later passes of that band stay
    there, so VectorE's in-order queue never waits on GpSimd."""
    if (unit, band) in PEH_BANDS:
        return "ddd"
    if band == 1:
        return "dppppp"
    if band == 3 and unit < 3:
        return "dddppp"
    return "dddddd"


# bands whose horizontal passes are emitted at the START of the next unit's
# section: they have no conv dependency left, so they fill the boundary
# bubble where VectorE/GpSimd would otherwise wait for the next unit's
# first psum copies
_DEFER_H = {3, 4}



def _build_nc():
    nc = bacc.Bacc("TRN2", target_bir_lowering=False)

    # x arrives host-padded to 66x66 (zero border) and channel-major
    # [n, icx, ch, h*w] so each partition's DMA run is contiguous
    x_d = nc.dram_tensor("x", [IMGS_PER_CORE, 2, CH, XFLAT], F32R, kind="ExternalInput")
    # Pre-arranged conv taps: [ic_part, icx, tap(a*3+b), ocx, oc]
    w_d = nc.dram_tensor("wt", [CH, 2, 9, 2, CH], F32R, kind="ExternalInput")
    # Diagonal FIR weights, fp16: [:,0:128] = I, [:,128:256] = 3I
    d_d = nc.dram_tensor("dg", [CH, 2 * CH], F16, kind="ExternalInput")
    # channel-major fp16 output [n, ocx, ch, r, s]; host transposes to NHWC
    y_d = nc.dram_tensor("y", [IMGS_PER_CORE, 2, CH, OUT, OUT], F16, kind="ExternalOutput")

    with tile.TileContext(nc) as tc:
        with (
            tc.tile_pool(name="const", bufs=1) as constp,
            tc.tile_pool(name="xp", bufs=2) as xp,
            tc.tile_pool(name="y1p", bufs=1) as y1p,
            tc.tile_pool(name="firp", bufs=3) as firp,
            tc.tile_pool(name="zp", bufs=4) as zp,
            tc.tile_pool(name="zdp", bufs=5) as zdp,
            tc.tile_pool(name="outp", bufs=3) as outp,
            tc.tile_pool(name="cpsum", bufs=5, space="PSUM") as cpsum,
            tc.tile_pool(name="fpsum", bufs=3, space="PSUM") as fpsum,
        ):
            w_sb = constp.tile([CH, 2, 9, 2, CH], F32R)
            # ocx=0 taps first so unit 0's matmuls can start sooner
            nc.sync.dma_start(out=w_sb[:, :, :, 0, :], in_=w_d[:, :, :, 0, :])

            def boxadd(eng, out, in0, in1):
                e = nc.vector if eng == "d" else nc.gpsimd
                e.tensor_tensor(out=out, in0=in0, in1=in1, op=ADD)

            # prefetch all images up front (fresh buffers, no WAR) so the
            # in-order SP queue never delays image 1 behind output DMAs
            x_sbs = []
            for n in range(IMGS_PER_CORE):
                # flat x image + 2 slack elems so full-row matmul spans with
                # a column offset stay in bounds (fp32r matmuls need
                # 2D-collapsible APs: contiguous spans covering whole rows)
                x_sb = xp.tile([CH, 2, XFLAT + 2], F32R, tag="x")
                nc.vector.memset(x_sb[:, 0, XFLAT : XFLAT + 2].bitcast(F32), 0.0)
                nc.vector.memset(x_sb[:, 1, XFLAT : XFLAT + 2].bitcast(F32), 0.0)
                for icx in range(2):
                    # two half-image transfers so the first conv chunks can
                    # start as soon as the top rows land
                    hx = 33 * XROW
                    nc.sync.dma_start(
                        out=x_sb[:, icx, 0:hx],
                        in_=x_d[n, icx, :, 0:hx],
                    )
                    nc.sync.dma_start(
                        out=x_sb[:, icx, hx:XFLAT],
                        in_=x_d[n, icx, :, hx:XFLAT],
                    )
                x_sbs.append(x_sb)
                if n == 0:
                    # remaining constants after unit 0's critical inputs
                    nc.sync.dma_start(
                        out=w_sb[:, :, :, 1, :], in_=w_d[:, :, :, 1, :]
                    )
                    dg_sb = constp.tile([CH, 2 * CH], F16)
                    nc.sync.dma_start(out=dg_sb[:], in_=d_d[:])

            # single y1 buffer reused by all units; the zero halo rows are
            # never overwritten, so set them once
            y1_sb = y1p.tile([CH, UP + 3, UP + 1], F16, tag="y1")
            nc.vector.memset(y1_sb[:, 0:1, 0:UP], 0.0)
            nc.vector.memset(y1_sb[:, UP + 1 : UP + 3, 0:UP], 0.0)

            # deferred PE horizontal-FIR bands: (n, ocx, r0, z tile)
            pending = []

            def flush_peh(count):
                for _ in range(min(count, len(pending))):
                    fn_, focx, fr0, fz = pending.pop(0)
                    out_sb = outp.tile([CH, BAND, OUT], F16, tag="out")
                    for g0 in range(0, BAND, GROUP):
                        ps2 = fpsum.tile([CH, GROUP, OUT], F32, tag="fps")
                        for v in range(4):
                            dgi = 0 if v in (0, 3) else 1
                            nc.tensor.matmul(
                                ps2[:],
                                lhsT=dg_sb[:, dgi * CH : (dgi + 1) * CH],
                                rhs=fz[:, g0 : g0 + GROUP, v + 1 : v + 1 + OUT],
                                start=(v == 0),
                                stop=(v == 3),
                            )
                        nc.scalar.copy(
                            out=out_sb[:, g0 : g0 + GROUP, :],
                            in_=ps2[:],
                        )
                    nc.sync.dma_start(
                        out=y_d[fn_, focx, :, fr0 : fr0 + BAND, :],
                        in_=out_sb[:],
                    )

            # horizontal FIR box passes for one band + output DMA
            def emit_h(pat, z, hn, hocx, hr0):
                h1 = firp.tile([CH, BAND, UP + 1], F16, tag="A")
                boxadd(
                    pat[3],
                    h1[:, :, 0 : UP + 1],
                    z[:, :, 1 : UP + 2],
                    z[:, :, 2 : UP + 3],
                )
                h2 = firp.tile([CH, BAND, UP], F16, tag="B")
                boxadd(
                    pat[4],
                    h2[:, :, 0:UP],
                    h1[:, :, 0:UP],
                    h1[:, :, 1 : UP + 1],
                )
                out_sb = outp.tile([CH, BAND, OUT], F16, tag="out")
                boxadd(
                    pat[5],
                    out_sb[:],
                    h2[:, :, 0:OUT],
                    h2[:, :, 1 : OUT + 1],
                )
                nc.sync.dma_start(
                    out=y_d[hn, hocx, :, hr0 : hr0 + BAND, :],
                    in_=out_sb[:],
                )

            deferred_h = []

            for n in range(IMGS_PER_CORE):
                x_sb = x_sbs[n]
                for ocx in range(2):
                    unit = 2 * n + ocx
                    # boundary-bubble filler: the previous unit's deferred
                    # horizontal bands (no conv dependency left)
                    for args in deferred_h:
                        emit_h(*args)
                    deferred_h.clear()
                    # ---------------- stage 1: conv into y1 (fp16) ----------
                    # y1_sb rows: up-row p at index p+1 (rows 0,130,131 zero)
                    # cols: up-col q at index q (col 129 pad, never read)
                    # band-major over up-row chunks: all 4 parities per
                    # m-chunk so the FIR bands can start while later rows
                    # are still being computed
                    for ci, m0 in enumerate(range(0, 65, 7)):
                        if ci >= _FLUSH_START:
                            flush_peh(1)
                        for rho in range(2):
                            for sig in range(2):
                                nm, nn = 65 - rho, 65 - sig
                                if m0 >= nm:
                                    continue
                                dis = (0, 1) if rho == 0 else (0,)
                                djs = (0, 1) if sig == 0 else (0,)
                                r = min(7, nm - m0)
                                ps = cpsum.tile([CH, r, XROW], F32, tag="cps")
                                # icx-major accumulation order: the first
                                # matmuls only need x[icx=0], overlapping
                                # with the x[icx=1] DMA on unit 0
                                mms = [
                                    (di, dj, icx2)
                                    for icx2 in range(2)
                                    for di in dis
                                    for dj in djs
                                ]
                                for k, (di, dj, icx2) in enumerate(mms):
                                    t = (2 * di + rho) * 3 + (2 * dj + sig)
                                    st = (m0 + 1 - di) * XROW + (1 - dj)
                                    nc.tensor.matmul(
                                        ps[:, 0:r, 0:XROW].opt({0}),
                                        lhsT=w_sb[:, icx2, t, ocx, :],
                                        rhs=x_sb[:, icx2, st : st + r * XROW],
                                        start=(k == 0),
                                        stop=(k == len(mms) - 1),
                                    )
                                # strided parity write into the up-grid
                                # (cols nn..65 of each psum row are garbage
                                # from the full-row span and are skipped)
                                nc.scalar.copy(
                                    out=y1_sb[
                                        :,
                                        1 + rho + 2 * m0 : 1 + rho + 2 * (m0 + r) : 2,
                                        sig : sig + 2 * nn : 2,
                                    ],
                                    in_=ps[:, 0:r, 0:nn],
                                )

                    # ---------------- stage 2: FIR box passes per band ------
                    # Software-pipelined emission: band k's horizontal
                    # passes are emitted after band k+2's vertical passes,
                    # so when a vertical pass is blocked on the conv
                    # frontier the in-order queues still have ready work.
                    hqueue = []
                    for bi, r0 in enumerate(range(0, OUT, BAND)):
                        pe_h = (unit, bi) in PEH_BANDS
                        pat = _pat(unit, bi)
                        # vertical: z[r] = y1[r-1] + 3 y1[r] + 3 y1[r+1] + y1[r+2]
                        # (up-row p at y1 index p+1)
                        b1 = firp.tile([CH, BAND + 2, UP + 1], F16, tag="A")
                        boxadd(
                            pat[0],
                            b1[:, :, 0:UP],
                            y1_sb[:, r0 : r0 + BAND + 2, 0:UP],
                            y1_sb[:, r0 + 1 : r0 + BAND + 3, 0:UP],
                        )
                        b2 = firp.tile([CH, BAND + 1, UP + 1], F16, tag="B")
                        boxadd(
                            pat[1],
                            b2[:, :, 0:UP],
                            b1[:, 0 : BAND + 1, 0:UP],
                            b1[:, 1 : BAND + 2, 0:UP],
                        )
                        # z cols: up-col q at index q+2 (idx 1 and 131 zero)
                        zpool = zdp if pe_h else zp
                        z = zpool.tile([CH, BAND, UP + 3], F16, tag="z")
                        nc.vector.memset(z[:, :, 1:2], 0.0)
                        nc.vector.memset(z[:, :, UP + 2 : UP + 3], 0.0)
                        boxadd(
                            pat[2],
                            z[:, :, 2 : UP + 2],
                            b2[:, 0:BAND, 0:UP],
                            b2[:, 1 : BAND + 1, 0:UP],
                        )

                        if pe_h:
                            # horizontal FIR on TensorE, deferred into the
                            # next unit's conv chunks (or the schedule tail)
                            pending.append((n, ocx, r0, z))
                        elif bi in _DEFER_H and unit < 3:
                            deferred_h.append((pat, z, n, ocx, r0))
                        else:
                            hqueue.append((pat, z, n, ocx, r0))
                        if bi >= 2 and len(hqueue) > 0 and hqueue[0][1] is not z:
                            emit_h(*hqueue.pop(0))
                    for args in hqueue:
                        emit_h(*args)
            # schedule tail: any deferred bands, then the last unit's PE
            # horizontal-FIR bands
            for args in deferred_h:
                emit_h(*args)
            deferred_h.clear()
            flush_peh(len(pending))
    nc.compile()
    return nc


_NC_CACHE = None


def _get_nc():
    global _NC_CACHE
    if _NC_CACHE is None:
        _NC_CACHE = _build_nc()
    return _NC_CACHE


def kernel(x, w, b):
    x = np.asarray(x, dtype=np.float32)
    w = np.asarray(w, dtype=np.float32)
    b = np.asarray(b, dtype=np.float32)
    # channel-major + zero pad: [N, 2, CH, (H+2)*(W+2)]
    xt = np.zeros((x.shape[0], 2, CH, H + 2, W + 2), dtype=np.float32)
    xt[:, :, :, 1 : H + 1, 1 : W + 1] = x.transpose(0, 3, 1, 2).reshape(
        x.shape[0], 2, CH, H, W
    )
    xt = xt.reshape(x.shape[0], 2, CH, XFLAT)

    # Effective transpose-conv filter, with the separable FIR normalisation
    # (1/4 per axis) folded in.
    Wf = w[::-1, ::-1] / 16.0  # [a, b, ic, oc]
    Wr = Wf.reshape(3, 3, 2, CH, 2, CH)  # a, b, icx, ic, ocx, oc
    w_arr = np.ascontiguousarray(
        Wr.transpose(3, 2, 0, 1, 4, 5).reshape(CH, 2, 9, 2, CH)
    )
    eye = np.eye(CH, dtype=np.float16)
    dg = np.ascontiguousarray(np.concatenate([eye, 3.0 * eye], axis=1))

    in_maps = [
        {
            "x": np.ascontiguousarray(xt[IMGS_PER_CORE * c : IMGS_PER_CORE * (c + 1)]),
            "wt": w_arr,
            "dg": dg,
        }
        for c in range(N_CORES)
    ]
    nc = _get_nc()
    res = run_bass_kernel_spmd(nc, in_maps, core_ids=list(range(N_CORES)))
    # [n, 2, CH, r, s] fp16 -> [n, r, s, 2*CH] fp32 + bias
    y = np.concatenate([res.results[c]["y"] for c in range(N_CORES)], axis=0)
    y = y.reshape(-1, C, OUT, OUT).transpose(0, 2, 3, 1).astype(np.float32)
    y += b.reshape(1, 1, 1, C)
    return np.ascontiguousarray(y)


if __name__ == "__main__":
    rng = np.random.default_rng(0)
    x = rng.standard_normal((16, 64, 64, 256), dtype=np.float32)
    w = rng.standard_normal((3, 3, 256, 256), dtype=np.float32) * 0.02
    b = np.zeros((256,), dtype=np.float32)
    y = kernel(x, w, b)
    print("out:", y.shape, y.dtype)
